# revision 32
# baseline (speedup 1.0000x reference)
"""DPGN (gnn_message_passing) fused Trainium2 kernel.

Sharding: pure data parallel over meta-batch B=256 -> 8 cores x 32 samples.
Per core, samples run in 8 blocks of 4. The whole 2-generation DPGN step is
fused on-chip; only inputs/outputs touch HBM.

Layouts (per block of 4 samples b=0..3):
  vT         [128, 120]  point features: channel on partition, (b,i) on free
  d2         [128, 3600] pairwise sq-dists: (b,i,j) on free
  edge tiles [128, 240]  rows 32b+i (32-aligned), free (kk,j)
  dist feats [128, *]    row-group packed: rows 32b+c (c<25)

Runtime: the axon PJRT tunnel has a ~68ms fixed RTT and ~100MB/s streams,
which dominates wall time (HW exec is <8ms).  Hence:
  - the jitted shard_map executable is built ONCE and cached (the stock
    run_bass_kernel_spmd retraces+recompiles per call, ~500ms overhead);
  - replicated weights are content-hashed and live on-device across calls;
  - data inputs and outputs cross the wire as fp16 (tolerance gate 2e-2,
    fp16 wire costs ~1e-3), converted on-chip / on-host;
  - no donated zero output buffers (every output element is written);
  - g0 node_l2 (= -|vi-vj|^2 of the raw point_node input) is recomputed
    on the host in f64 while the device round trip is in flight, so the
    device ships 5 output channels instead of 6.
"""
import sys
sys.path.insert(0, "/opt/trn_rl_repo")
from contextlib import ExitStack

import numpy as np
import concourse.bass as bass
import concourse.bacc as bacc
import concourse.tile as tile
from concourse import mybir
from concourse.bass_utils import run_bass_kernel_spmd
from concourse.masks import make_identity

F32 = mybir.dt.float32
AF = mybir.ActivationFunctionType
OP = mybir.AluOpType
AX = mybir.AxisListType

G, B, N, S, D = 2, 256, 30, 25, 128
NCORES = 8
BC = B // NCORES          # 32 samples per core
NBLK = BC // 4            # 8 blocks of 4 samples
EW = NBLK * N             # 240
NEG = 0.01
BN_SCALE = float(1.0 / np.sqrt(1.0 + 1e-5))
EPS_L1 = 1e-12

# matmul operand mode: "f32" (exact, 4 cyc/row) | "f32r" (reduced-precision mul, 1 cyc/row)
MM_MODE = "f32"
# leaky-relu implementation: "act" (1 ScalarE op; not in CoreSim) | "dve" (Identity + DVE max)
LRELU_ON = "act"
# debug: comma set of enabled parts: "setup,p1,p2,p3,p4,p5" (default all)
import os as _os
PHASES = set((_os.environ.get("KPHASES") or "setup,p1,p2,p3,p4,p5").split(","))
KGENS = int(_os.environ.get("KGENS") or G)
KREPEAT = int(_os.environ.get("KREPEAT") or 1)

_NC_CACHE = {}


BF16 = mybir.dt.bfloat16
F32R = mybir.dt.float32r
F16 = mybir.dt.float16
# wire dtype for the 4 data inputs and the output (tunnel-bandwidth bound;
# tolerance gate is 2e-2, fp16 wire adds ~1e-3)
IO_DT = F16 if (_os.environ.get("KIODT") or "f16") == "f16" else F32
IO_NP = np.float16 if IO_DT == F16 else np.float32


def _dt_point():   # d2, h1, w1T, w2T (base-0 matmuls only)
    if MM_MODE == "hybrid":
        return F32R
    if MM_MODE == "bf16":
        return BF16
    return F32


def _dt_flex():    # dist chain (col/row-tiled matmuls)
    if MM_MODE in ("hybrid", "bf16"):
        return BF16
    return F32


def _dt_s():       # h2 / w3T (s-path: accuracy-sensitive)
    return BF16 if MM_MODE == "bf16" else F32


def _mm(ap):
    return ap


def A(t, ap, off=0):
    return bass.AP(tensor=t.tensor, offset=t.offset + off, ap=ap)


def build_nc():
    nc = bacc.Bacc("TRN2", target_bir_lowering=False, debug=False)
    MDP = _dt_point()
    MDF = _dt_flex()
    MDS = _dt_s()

    pn_d = nc.dram_tensor("point_node", [BC, N, D], IO_DT, kind="ExternalInput")
    pe_d = nc.dram_tensor("point_edge", [BC, N, N], IO_DT, kind="ExternalInput")
    dn_d = nc.dram_tensor("distribution_node", [BC, N, S], IO_DT, kind="ExternalInput")
    de_d = nc.dram_tensor("distribution_edge", [BC, N, N], IO_DT, kind="ExternalInput")
    wd = {}
    for name, shape in [
        ("ps_w1", [G, 2 * D, D]), ("ps_g1", [G, 2 * D]), ("ps_b1", [G, 2 * D]),
        ("ps_w2", [G, D, 2 * D]), ("ps_g2", [G, D]), ("ps_b2", [G, D]),
        ("ps_w3", [G, 1, D]), ("ps_b3", [G, 1]),
        ("p2d_w", [G, S, 2 * S]), ("p2d_b", [G, S]),
        ("ds_w1", [G, 2 * S, S]), ("ds_g1", [G, 2 * S]), ("ds_b1", [G, 2 * S]),
        ("ds_w2", [G, S, 2 * S]), ("ds_g2", [G, S]), ("ds_b2", [G, S]),
        ("ds_w3", [G, 1, S]), ("ds_b3", [G, 1]),
        ("dp_w1", [G, 2 * D, 2 * D]), ("dp_g1", [G, 2 * D]), ("dp_b1", [G, 2 * D]),
        ("dp_w2", [G, D, 2 * D]), ("dp_g2", [G, D]), ("dp_b2", [G, D]),
    ]:
        wd[name] = nc.dram_tensor(name, shape, F32, kind="ExternalInput")
    # output channels: 0=g0 point_edge, 1=g0 dist_edge, 2=g1 point_edge,
    # 3=g1 node_l2, 4=g1 dist_edge.  g0 node_l2 is recomputed on the host
    # from the raw point_node input (cheaper than shipping it).
    out_d = nc.dram_tensor("out", [5, BC, N, N], IO_DT, kind="ExternalOutput")
    OCH, OB = BC * N * N, N * N

    with tile.TileContext(nc) as tc, ExitStack() as ctx:
        cp = ctx.enter_context(tc.tile_pool(name="cpool", bufs=1))
        vp = ctx.enter_context(tc.tile_pool(name="vpool", bufs=1))
        wp = ctx.enter_context(tc.tile_pool(name="wpool", bufs=2))
        ep = ctx.enter_context(tc.tile_pool(name="epool", bufs=2))
        PB = ctx.enter_context(tc.tile_pool(name="PB", bufs=2, space="PSUM"))
        PM = ctx.enter_context(tc.tile_pool(name="PM", bufs=3, space="PSUM"))

        # ================= constants =================
        ident = cp.tile([128, 128], F32, tag="ident")
        make_identity(nc, ident[:])
        off_m = cp.tile([120, N], F32, tag="off_m")           # 1 - eye (30-stride)
        eyeeps = cp.tile([120, N], F32, tag="eyeeps")         # eye + 1e-6
        nc.gpsimd.memset(off_m[:], 1.0)
        nc.gpsimd.memset(eyeeps[:], 1e-6)
        for t, fill in ((off_m, 0.0), (eyeeps, 1.0 + 1e-6)):
            nc.gpsimd.affine_select(
                out=t[0:N, :], in_=t[0:N, :],
                compare_op=OP.not_equal, fill=fill, base=0,
                pattern=[[-1, N]], channel_multiplier=1)
            for b in range(1, 4):
                nc.sync.dma_start(out=t[30 * b:30 * b + N, :], in_=t[0:N, :])
        Eb = cp.tile([S, 4, 128], F32, tag="Eb")              # 1 at (c, 32b+c)
        nc.gpsimd.memset(Eb[:], 0.0)
        for b in range(4):
            nc.gpsimd.affine_select(
                out=Eb[:, b, :], in_=Eb[:, b, :], compare_op=OP.not_equal,
                fill=1.0, base=32 * b, pattern=[[-1, 128]], channel_multiplier=1)
        E2 = cp.tile([2 * S, 2, 128], F32, tag="E2")          # 1 at (c, 64q+c)
        nc.gpsimd.memset(E2[:], 0.0)
        for q in range(2):
            nc.gpsimd.affine_select(
                out=E2[:, q, :], in_=E2[:, q, :], compare_op=OP.not_equal,
                fill=1.0, base=64 * q, pattern=[[-1, 128]], channel_multiplier=1)
        onesT = cp.tile([128, 32], F32, tag="onesT")
        ones_f = cp.tile([128, 32], F32, tag="ones_f")
        nc.vector.memset(ones_f[:], 0.0)
        nc.vector.memset(ones_f[:, 0:1], 1.0)
        nc.vector.tensor_copy(onesT[:], ones_f[:])


        def act_lrelu(out_ap, in_ap, scale, bias):
            if LRELU_ON == "act":
                # Prelu == leaky relu; lives in the sigmoid table set (Lrelu does not,
                # and mixing Lrelu+Sigmoid table loads crashes the ACT engine)
                nc.scalar.activation(out=out_ap, in_=in_ap, func=AF.Prelu,
                                     alpha=NEG, scale=scale, bias=bias)
            elif LRELU_ON == "actsim":
                # timing-equivalent stand-in for CoreSim (values wrong: no lrelu)
                nc.scalar.activation(out=out_ap, in_=in_ap, func=AF.Identity,
                                     scale=scale, bias=bias)
            else:
                nc.scalar.activation(out=out_ap, in_=in_ap, func=AF.Identity,
                                     scale=scale, bias=bias)
                nc.vector.scalar_tensor_tensor(out=out_ap, in0=out_ap, scalar=NEG,
                                               in1=out_ap, op0=OP.mult, op1=OP.max)

        def load_col(name, g, n, tag, blocks=1, scale=None):
            t = cp.tile([128, blocks], F32, tag=tag)
            if blocks > 1:
                src = bass.AP(tensor=wd[name], offset=g * n * blocks,
                              ap=[[1, n], [n, blocks]])
                dst = A(t, [[blocks, n], [1, blocks]])
            else:
                src = bass.AP(tensor=wd[name], offset=g * n, ap=[[1, n]])
                dst = A(t, [[1, n], [1, 1]])
            nc.sync.dma_start(out=dst, in_=src)
            if scale is not None:
                nc.vector.tensor_scalar(out=t[:n, :], in0=t[:n, :], scalar1=scale,
                                        scalar2=None, op0=OP.mult)
            return t

        def load_col_rep(name, g, n, tag, bases, scale=None):
            t = cp.tile([128, 1], F32, tag=tag)
            nc.vector.memset(t[:], 0.0)
            src = bass.AP(tensor=wd[name], offset=g * n, ap=[[1, n], [1, 1]])
            for bb in bases:
                nc.sync.dma_start(out=t[bb:bb + n, :], in_=src)
            if scale is not None:
                for bb in bases:
                    nc.vector.tensor_scalar(out=t[bb:bb + n, :], in0=t[bb:bb + n, :],
                                            scalar1=scale, scalar2=None, op0=OP.mult)
            return t

        def transpose_to(dst_ap, src_ap, idn):
            p = src_ap.partition_size()
            f = src_ap.free_size()
            pt = PM.tile([128, 512], F32, tag="med")
            nc.tensor.transpose(pt[:f, :p], src_ap, idn)
            nc.vector.tensor_copy(dst_ap, pt[:f, :p])

        # ================= weights =================
        W = {g: {} for g in range(G)}
        for g in range(G):
            w = W[g]
            w1T = cp.tile([128, 2 * D], MDP, tag=f"w1T{g}")
            for h in range(2):
                tmp = wp.tile([128, D], F32, tag="wload")
                nc.sync.dma_start(out=tmp[:], in_=wd["ps_w1"][g, 128 * h:128 * (h + 1), :])
                transpose_to(w1T[:, 128 * h:128 * (h + 1)], tmp[:], ident[:])
            w["w1T"] = w1T
            w2T = cp.tile([128, 2, D], MDP, tag=f"w2T{g}")
            tmp = wp.tile([128, 2 * D], F32, tag="wload2")
            nc.sync.dma_start(out=tmp[:], in_=wd["ps_w2"][g])
            for k in range(2):
                transpose_to(w2T[:, k, :], tmp[:, 128 * k:128 * (k + 1)], ident[:])
            w["w2T"] = w2T
            w3T = cp.tile([128, 32], MDS, tag=f"w3T{g}")
            w3f = wp.tile([128, 32], F32, tag="wst")
            nc.vector.memset(w3f[:], 0.0)
            nc.sync.dma_start(out=A(w3f, [[32, 128], [1, 1]]),
                              in_=bass.AP(tensor=wd["ps_w3"], offset=g * D, ap=[[1, D]]))
            nc.vector.tensor_copy(w3T[:], w3f[:])
            w["w3T"] = w3T
            w["gs1"] = load_col("ps_g1", g, 128, f"gs1{g}", 2, scale=BN_SCALE)
            w["bs1"] = load_col("ps_b1", g, 128, f"bs1{g}", 2)
            w["gs2"] = load_col("ps_g2", g, 128, f"gs2{g}", scale=BN_SCALE)
            w["bs2"] = load_col("ps_b2", g, 128, f"bs2{g}")
            b3bc = cp.tile([128, 1], F32, tag=f"b3bc{g}")
            nc.sync.dma_start(out=b3bc[:],
                              in_=bass.AP(tensor=wd["ps_b3"], offset=g, ap=[[0, 128], [1, 1]]))
            w["b3bc"] = b3bc

            tmp = wp.tile([S, 2 * S], F32, tag="wload3")
            nc.sync.dma_start(out=tmp[:], in_=wd["p2d_w"][g])
            p2dA = cp.tile([S, 32], F32, tag=f"p2dA{g}")
            nc.vector.memset(p2dA[:], 0.0)
            transpose_to(p2dA[:, 0:S], tmp[:, 0:S], ident[:S, :S])
            p2dAr = cp.tile([128, 32], F32, tag=f"p2dAr{g}")
            nc.vector.memset(p2dAr[:], 0.0)
            ptA = PM.tile([128, 512], F32, tag="med")
            for b in range(4):
                nc.tensor.matmul(ptA[:, :32], Eb[:, b, :], p2dA[:],
                                 start=(b == 0), stop=(b == 3))
            nc.vector.tensor_copy(p2dAr[:, :], ptA[:, :32])
            w["p2dAr"] = p2dAr
            p2dBf = wp.tile([S, S], F32, tag="p2dBf")
            transpose_to(p2dBf[:], tmp[:, S:2 * S], ident[:S, :S])
            p2dB = cp.tile([128, 32], F32, tag=f"p2dB{g}")
            nc.vector.memset(p2dB[:], 0.0)
            pt = PM.tile([128, 512], F32, tag="med")
            for b in range(4):
                nc.tensor.matmul(pt[:, :S], Eb[:, b, :], p2dBf[:],
                                 start=(b == 0), stop=(b == 3))
            nc.vector.tensor_copy(p2dB[:, 0:S], pt[:, :S])
            w["p2dA"], w["p2dB"] = p2dA, p2dB
            w["p2db"] = load_col_rep("p2d_b", g, S, f"p2db{g}", [0, 32, 64, 96])

            tmp = wp.tile([2 * S, S], F32, tag="wload4")
            nc.sync.dma_start(out=tmp[:], in_=wd["ds_w1"][g])
            dsw1f = wp.tile([S, 2 * S], F32, tag="dsw1f")
            transpose_to(dsw1f[:], tmp[:], ident[:2 * S, :2 * S])
            dsw1 = cp.tile([128, 64], MDF, tag=f"dsw1{g}")
            d1f = wp.tile([128, 64], F32, tag="wst2")
            nc.vector.memset(d1f[:], 0.0)
            pt = PM.tile([128, 512], F32, tag="med")
            for b in range(4):
                nc.tensor.matmul(pt[:, :2 * S], Eb[:, b, :], dsw1f[:],
                                 start=(b == 0), stop=(b == 3))
            nc.vector.tensor_copy(d1f[:, 0:2 * S], pt[:, :2 * S])
            nc.vector.tensor_copy(dsw1[:], d1f[:])
            w["dsw1"] = dsw1
            tmp = wp.tile([S, 2 * S], F32, tag="wload5")
            nc.sync.dma_start(out=tmp[:], in_=wd["ds_w2"][g])
            dsw2f = wp.tile([2 * S, S], F32, tag="dsw2f")
            transpose_to(dsw2f[:], tmp[:], ident[:S, :S])
            dsw2 = cp.tile([128, 32], MDF, tag=f"dsw2{g}")
            d2f = wp.tile([128, 32], F32, tag="wst3")
            nc.vector.memset(d2f[:], 0.0)
            pt = PM.tile([128, 512], F32, tag="med")
            for q in range(2):
                nc.tensor.matmul(pt[:, :S], E2[:, q, :], dsw2f[:],
                                 start=(q == 0), stop=(q == 1))
            nc.vector.tensor_copy(d2f[:, 0:S], pt[:, :S])
            nc.vector.tensor_copy(dsw2[:], d2f[:])
            w["dsw2"] = dsw2
            dsw3 = cp.tile([128, 32], MDF, tag=f"dsw3{g}")
            d3f = wp.tile([128, 32], F32, tag="wst4")
            nc.vector.memset(d3f[:], 0.0)
            for b in range(4):
                nc.sync.dma_start(out=d3f[32 * b:32 * b + S, 0:1],
                                  in_=bass.AP(tensor=wd["ds_w3"], offset=g * S, ap=[[1, S], [1, 1]]))
            nc.vector.tensor_copy(dsw3[:], d3f[:])
            w["dsw3"] = dsw3
            w["dsg1"] = load_col_rep("ds_g1", g, 2 * S, f"dsg1{g}", [0, 64], scale=BN_SCALE)
            w["dsb1"] = load_col_rep("ds_b1", g, 2 * S, f"dsb1{g}", [0, 64])
            w["dsg2"] = load_col_rep("ds_g2", g, S, f"dsg2{g}", [0, 32, 64, 96], scale=BN_SCALE)
            w["dsb2"] = load_col_rep("ds_b2", g, S, f"dsb2{g}", [0, 32, 64, 96])
            dsb3bc = cp.tile([128, 1], F32, tag=f"dsb3bc{g}")
            nc.sync.dma_start(out=dsb3bc[:],
                              in_=bass.AP(tensor=wd["ds_b3"], offset=g, ap=[[0, 128], [1, 1]]))
            w["dsb3bc"] = dsb3bc

            if g < G - 1:
                dpw1T = [cp.tile([128, 2 * D], F32, tag=f"dpw1T{g}_{k}", name=f"dpw1T{g}_{k}") for k in range(2)]
                for r in range(2):
                    tmp = wp.tile([128, 2 * D], F32, tag="wload6")
                    nc.sync.dma_start(out=tmp[:], in_=wd["dp_w1"][g, 128 * r:128 * (r + 1), :])
                    for k in range(2):
                        transpose_to(dpw1T[k][:, 128 * r:128 * (r + 1)],
                                     tmp[:, 128 * k:128 * (k + 1)], ident[:])
                w["dpw1T"] = dpw1T
                tmp = wp.tile([128, 2 * D], F32, tag="wload7")
                nc.sync.dma_start(out=tmp[:], in_=wd["dp_w2"][g])
                dpw2T = [cp.tile([128, D], F32, tag=f"dpw2T{g}_{k}", name=f"dpw2T{g}_{k}") for k in range(2)]
                for k in range(2):
                    transpose_to(dpw2T[k][:], tmp[:, 128 * k:128 * (k + 1)], ident[:])
                w["dpw2T"] = dpw2T
                w["dpg1"] = load_col("dp_g1", g, 128, f"dpg1{g}", 2, scale=BN_SCALE)
                w["dpb1"] = load_col("dp_b1", g, 128, f"dpb1{g}", 2)
                w["dpg2"] = load_col("dp_g2", g, 128, f"dpg2{g}", scale=BN_SCALE)
                w["dpb2"] = load_col("dp_b2", g, 128, f"dpb2{g}")

        # ================= persistent state =================
        vT = [vp.tile([128, BC * N], F32, tag=f"vT{i}", name=f"vT{i}") for i in range(2)]
        dn_rg = vp.tile([128, EW], F32, tag="dn_rg")
        pe_all = vp.tile([120, EW], F32, tag="pe_all")
        de_all = vp.tile([120, EW], F32, tag="de_all")
        s_all = vp.tile([120, EW], F32, tag="s_all")
        sds_all = vp.tile([120, EW], F32, tag="sds_all")
        ef_all = vp.tile([120, EW], F32, tag="ef_all")
        for t in (pe_all, de_all, s_all, sds_all, ef_all, dn_rg, vT[0], vT[1]):
            nc.gpsimd.memset(t[:], 0.0)

        # ---- gen-1 input staging ----
        for kk in range(NBLK):
            pf = wp.tile([120, D], F32, tag="pnflat")
            if IO_DT == F32:
                nc.sync.dma_start(out=pf[:], in_=pn_d[4 * kk:4 * (kk + 1)].rearrange("b n d -> (b n) d"))
            else:
                pf16 = wp.tile([120, D], IO_DT, tag="pnflat16")
                nc.sync.dma_start(out=pf16[:], in_=pn_d[4 * kk:4 * (kk + 1)].rearrange("b n d -> (b n) d"))
                nc.vector.tensor_copy(pf[:], pf16[:])
            pt = PM.tile([128, 512], F32, tag="med")
            nc.tensor.transpose(pt[:, :120], pf[:], ident[:120, :120])
            nc.vector.tensor_copy(vT[0][:, 120 * kk:120 * (kk + 1)], pt[:, :120])

            df = wp.tile([120, S], F32, tag="dnflat")
            if IO_DT == F32:
                nc.sync.dma_start(out=df[:], in_=dn_d[4 * kk:4 * (kk + 1)].rearrange("b n s -> (b n) s"))
            else:
                df16 = wp.tile([120, S], IO_DT, tag="dnflat16")
                nc.sync.dma_start(out=df16[:], in_=dn_d[4 * kk:4 * (kk + 1)].rearrange("b n s -> (b n) s"))
                nc.vector.tensor_copy(df[:], df16[:])
            pt2 = PM.tile([128, 512], F32, tag="med")
            nc.tensor.transpose(pt2[:S, :120], df[:], ident[:120, :120])
            dnf = wp.tile([S, 120], F32, tag="dnf")
            nc.vector.tensor_copy(dnf[:], pt2[:S, :120])
            pt3 = PM.tile([128, 512], F32, tag="med")
            for b in range(4):
                nc.tensor.matmul(pt3[:, :N], Eb[:, b, :], dnf[:, 30 * b:30 * b + N],
                                 start=(b == 0), stop=(b == 3))
            nc.vector.tensor_copy(dn_rg[:, N * kk:N * (kk + 1)], pt3[:, :N])

            for (ed, et) in ((pe_d, pe_all), (de_d, de_all)):
                if IO_DT == F32:
                    nc.sync.dma_start(out=et[:, N * kk:N * (kk + 1)],
                                      in_=ed[4 * kk:4 * (kk + 1)].rearrange("b n m -> (b n) m"))
                else:
                    e16 = wp.tile([120, N], IO_DT, tag="edge16")
                    nc.sync.dma_start(out=e16[:],
                                      in_=ed[4 * kk:4 * (kk + 1)].rearrange("b n m -> (b n) m"))
                    nc.vector.tensor_copy(et[:, N * kk:N * (kk + 1)], e16[:])

        def edge_update(g, w, e_all, sig_src, b3bc, out_ch):
            ssig = ep.tile([120, EW], F32, tag="ssig")
            nc.scalar.activation(out=ssig[:], in_=sig_src[:], func=AF.Sigmoid,
                                 bias=b3bc[:120, :], scale=1.0)
            em = ep.tile([120, EW], F32, tag="em")
            offb = A(off_m, [[N, 120], [0, NBLK], [1, N]])
            emv = A(em, [[EW, 120], [N, NBLK], [1, N]])
            nc.vector.tensor_tensor(out=emv, in0=A(e_all, [[EW, 120], [N, NBLK], [1, N]]),
                                    in1=offb, op=OP.mult)
            esum = ep.tile([120, NBLK], F32, tag="esum")
            nc.vector.tensor_reduce(out=esum[:], in_=emv, axis=AX.X, op=OP.add)
            t = ep.tile([120, EW], F32, tag="t")
            nc.vector.tensor_tensor(out=t[:], in0=ssig[:], in1=em[:], op=OP.mult)
            ts = ep.tile([120, NBLK], F32, tag="ts")
            nc.vector.tensor_reduce(out=ts[:], in_=A(t, [[EW, 120], [N, NBLK], [1, N]]),
                                    axis=AX.X, op=OP.add)
            nc.vector.tensor_scalar(out=ts[:], in0=ts[:], scalar1=EPS_L1,
                                    scalar2=None, op0=OP.max)
            r = ep.tile([120, NBLK], F32, tag="r")
            nc.vector.reciprocal(out=r[:], in_=ts[:])
            nc.vector.tensor_tensor(out=r[:], in0=r[:], in1=esum[:], op=OP.mult)
            e2 = ep.tile([120, EW], F32, tag="e2")
            rb = A(r, [[NBLK, 120], [1, NBLK], [0, N]])
            e2v = A(e2, [[EW, 120], [N, NBLK], [1, N]])
            nc.vector.tensor_tensor(out=e2v, in0=A(t, [[EW, 120], [N, NBLK], [1, N]]),
                                    in1=rb, op=OP.mult)
            eyb = A(eyeeps, [[N, 120], [0, NBLK], [1, N]])
            nc.vector.tensor_tensor(out=e2v, in0=e2v, in1=eyb, op=OP.add)
            rsum = ep.tile([120, NBLK], F32, tag="rsum")
            nc.vector.tensor_reduce(out=rsum[:], in_=e2v, axis=AX.X, op=OP.add)
            rr = ep.tile([120, NBLK], F32, tag="rr")
            nc.vector.reciprocal(out=rr[:], in_=rsum[:])
            rrb = A(rr, [[NBLK, 120], [1, NBLK], [0, N]])
            nc.vector.tensor_tensor(out=A(e_all, [[EW, 120], [N, NBLK], [1, N]]),
                                    in0=e2v, in1=rrb, op=OP.mult)
            if IO_DT == F32:
                esrc = e_all
            else:
                esrc = ep.tile([120, EW], IO_DT, tag="eo16")
                nc.vector.tensor_copy(esrc[:], e_all[:])
            for kk in range(NBLK):
                dst = bass.AP(tensor=out_d,
                              offset=out_ch * OCH + 4 * kk * OB,
                              ap=[[N, 120], [1, N]])
                nc.sync.dma_start(out=dst, in_=esrc[:, N * kk:N * (kk + 1)])

        PSUM_PAT = [[1024, 128], [512, 2], [1, 450]]

        # ================= generations =================
        for _rep in range(KREPEAT):
         for g in range(KGENS):
            w = W[g]
            vc, vn = vT[g % 2], vT[(g + 1) % 2]

            # ---------- phase 1: point sim MLP ----------
            for kk in range(NBLK if "p1" in PHASES else 0):
                base = 120 * kk
                d2 = wp.tile([128, 4 * N * N], MDP, tag="d2")
                vi = A(vc, [[BC * N, 128], [N, 4], [1, N], [0, N]], off=base)
                vj = A(vc, [[BC * N, 128], [N, 4], [0, N], [1, N]], off=base)
                dv = A(d2, [[3600, 128], [900, 4], [N, N], [1, N]])
                nc.vector.tensor_tensor(out=dv, in0=vi, in1=vj, op=OP.subtract)
                nc.vector.tensor_tensor(out=d2[:], in0=d2[:], in1=d2[:], op=OP.mult)
                h2 = wp.tile([128, 4 * N * N], MDS, tag="h2")
                for bb in range(4):   # per sample
                    h1 = [wp.tile([128, N * N], MDP, tag=f"h1_{h}", name=f"h1_{h}") for h in range(2)]
                    for h in range(2):
                        pb = PB.tile([128, 2, 512], F32, tag="big")
                        for p in range(2):
                            nc.tensor.matmul(pb[:, p, 0:450],
                                             _mm(w["w1T"][:, 128 * h:128 * (h + 1)]),
                                             _mm(d2[:, 900 * bb + 450 * p:900 * bb + 450 * (p + 1)]),
                                             start=True, stop=True)
                        act_lrelu(A(h1[h], [[900, 128], [450, 2], [1, 450]]),
                                  A(pb, PSUM_PAT),
                                  w["gs1"][:, h:h + 1], w["bs1"][:, h:h + 1])
                    pb = PB.tile([128, 2, 512], F32, tag="big")
                    for p in range(2):
                        for k in range(2):
                            nc.tensor.matmul(pb[:, p, 0:450],
                                             _mm(w["w2T"][:, k, :]),
                                             _mm(h1[k][:, 450 * p:450 * (p + 1)]),
                                             start=(k == 0), stop=(k == 1))
                    act_lrelu(A(h2, [[3600, 128], [450, 2], [1, 450]], off=900 * bb),
                              A(pb, PSUM_PAT), w["gs2"][:], w["bs2"][:])
                # s_pre and node_l2 via col-tiled M=1 matmuls
                # (g0 node_l2 is host-computed from raw point_node — skip stage 1)
                for stage in range(2 if g == 1 else 1):
                    rhs_t, lhs = (h2, w["w3T"]) if stage == 0 else (d2, onesT)
                    pb = PB.tile([128, 2, 512], F32, tag="big")
                    for p in range(2):
                        for b in range(4):
                            rr = rhs_t[:, 900 * b + 450 * p:900 * b + 450 * (p + 1)]
                            if stage == 1 and rr.dtype == F32R:
                                rr = rr.bitcast(F32)
                            nc.tensor.matmul(
                                pb[32 * b:32 * b + 32, p, 0:450],
                                lhs[:], rr,
                                start=True, stop=True, tile_position=(0, 32 * b))
                    stg = wp.tile([128, 900], F32, tag=f"stg{stage}")
                    if stage == 0:
                        nc.vector.tensor_copy(A(stg, [[900, 128], [450, 2], [1, 450]]),
                                              A(pb, PSUM_PAT))
                        src = A(stg, [[32 * 900, 4], [N, N], [1, N]])
                        nc.sync.dma_start(out=s_all[:, N * kk:N * (kk + 1)], in_=src)
                    else:
                        if IO_DT != F32:
                            stgo = wp.tile([128, 900], IO_DT, tag="stg1o", name="stgo")
                        else:
                            stgo = stg
                        nc.vector.tensor_scalar(
                            out=A(stgo, [[900, 128], [450, 2], [1, 450]]),
                            in0=A(pb, PSUM_PAT),
                            scalar1=-1.0, scalar2=None, op0=OP.mult)
                        for b in range(4):
                            src = A(stgo, [[900, 1], [N, N], [1, N]], off=32 * b * 900)
                            dst = bass.AP(tensor=out_d,
                                          offset=3 * OCH + (4 * kk + b) * OB,
                                          ap=[[N, N], [1, N]])
                            nc.sync.dma_start(out=dst, in_=src)

            # ---------- phase 2: point edge update ----------
            if "p2" in PHASES:
                edge_update(g, w, pe_all, s_all, w["b3bc"], 0 if g == 0 else 2)

            # ---------- phase 3: p2d + dist sim ----------
            for kk in range(NBLK if "p3" in PHASES else 0):
                peT = wp.tile([S, 120], F32, tag="peT")
                pt = PM.tile([128, 512], F32, tag="med")
                nc.tensor.transpose(pt[:S, :120], pe_all[:, N * kk:N * kk + S],
                                    ident[:120, :120])
                nc.vector.tensor_copy(peT[:], pt[:S, :120])
                ptg = PM.tile([128, 512], F32, tag="med")
                for b in range(4):
                    nc.tensor.matmul(ptg[:, :N], Eb[:, b, :],
                                     peT[:, 30 * b:30 * b + N],
                                     start=(b == 0), stop=(b == 3))
                peRG = wp.tile([128, N], F32, tag="peRG")
                nc.vector.tensor_copy(peRG[:], ptg[:, :N])
                pg = PM.tile([128, 512], F32, tag="med")
                for b in range(4):
                    nc.tensor.matmul(pg[32 * b:32 * b + 32, :N],
                                     _mm(w["p2dAr"][32 * b:32 * b + S, :]),
                                     _mm(peRG[32 * b:32 * b + S, :]),
                                     start=True, stop=False, tile_position=(32 * b, 32 * b))
                    nc.tensor.matmul(pg[32 * b:32 * b + 32, :N],
                                     _mm(w["p2dB"][32 * b:32 * b + S, :]),
                                     _mm(dn_rg[32 * b:32 * b + S, N * kk:N * (kk + 1)]),
                                     start=False, stop=True, tile_position=(32 * b, 32 * b))
                act_lrelu(dn_rg[:, N * kk:N * (kk + 1)], pg[:, :N], 1.0, w["p2db"][:])
                dd2 = wp.tile([128, N * N], MDF, tag="dd2")
                vi = A(dn_rg, [[EW, 128], [1, N], [0, N]], off=N * kk)
                vj = A(dn_rg, [[EW, 128], [0, N], [1, N]], off=N * kk)
                nc.vector.tensor_tensor(out=A(dd2, [[900, 128], [N, N], [1, N]]),
                                        in0=vi, in1=vj, op=OP.subtract)
                nc.vector.tensor_tensor(out=dd2[:], in0=dd2[:], in1=dd2[:], op=OP.mult)
                h1d = [wp.tile([128, N * N], MDF, tag=f"h1d{p}", name=f"h1d{p}") for p in range(2)]
                for pair in range(2):
                    pb = PB.tile([128, 2, 512], F32, tag="big")
                    for ck in range(2):
                        for q in range(2):
                            b = 2 * pair + q
                            nc.tensor.matmul(
                                pb[64 * q:64 * q + 64, ck, 0:450],
                                _mm(w["dsw1"][32 * b:32 * b + S, :]),
                                _mm(dd2[32 * b:32 * b + S, 450 * ck:450 * (ck + 1)]),
                                start=True, stop=True, tile_position=(32 * b, 64 * q))
                    act_lrelu(A(h1d[pair], [[900, 128], [450, 2], [1, 450]]),
                              A(pb, PSUM_PAT), w["dsg1"][:], w["dsb1"][:])
                h2d = wp.tile([128, N * N], MDF, tag="h2d")
                pb = PB.tile([128, 2, 512], F32, tag="big")
                for ck in range(2):
                    for pair in range(2):
                        for q in range(2):
                            b = 2 * pair + q
                            nc.tensor.matmul(
                                pb[32 * b:32 * b + 32, ck, 0:450],
                                _mm(w["dsw2"][64 * q:64 * q + 2 * S, :]),
                                _mm(h1d[pair][64 * q:64 * q + 2 * S, 450 * ck:450 * (ck + 1)]),
                                start=True, stop=True, tile_position=(64 * q, 32 * b))
                act_lrelu(A(h2d, [[900, 128], [450, 2], [1, 450]]),
                          A(pb, PSUM_PAT), w["dsg2"][:], w["dsb2"][:])
                pb = PB.tile([128, 2, 512], F32, tag="big")
                for ck in range(2):
                    for b in range(4):
                        nc.tensor.matmul(
                            pb[32 * b:32 * b + 32, ck, 0:450],
                            _mm(w["dsw3"][32 * b:32 * b + S, :]),
                            _mm(h2d[32 * b:32 * b + S, 450 * ck:450 * (ck + 1)]),
                            start=True, stop=True, tile_position=(32 * b, 32 * b))
                stg = wp.tile([128, 900], F32, tag="stgd")
                nc.vector.tensor_copy(A(stg, [[900, 128], [450, 2], [1, 450]]),
                                      A(pb, PSUM_PAT))
                src = A(stg, [[32 * 900, 4], [N, N], [1, N]])
                nc.sync.dma_start(out=sds_all[:, N * kk:N * (kk + 1)], in_=src)

            # ---------- phase 4: dist edge update (+ ef) ----------
            if "p4" in PHASES:
                edge_update(g, w, de_all, sds_all, w["dsb3bc"], 1 if g == 0 else 4)
            if g < G - 1 and "p5" in PHASES:
                em2 = ep.tile([120, EW], F32, tag="em2")
                offb = A(off_m, [[N, 120], [0, NBLK], [1, N]])
                em2v = A(em2, [[EW, 120], [N, NBLK], [1, N]])
                nc.vector.tensor_tensor(out=em2v,
                                        in0=A(de_all, [[EW, 120], [N, NBLK], [1, N]]),
                                        in1=offb, op=OP.mult)
                s2 = ep.tile([120, NBLK], F32, tag="s2")
                nc.vector.tensor_reduce(out=s2[:], in_=em2v, axis=AX.X, op=OP.add)
                nc.vector.tensor_scalar(out=s2[:], in0=s2[:], scalar1=EPS_L1,
                                        scalar2=None, op0=OP.max)
                r2 = ep.tile([120, NBLK], F32, tag="r2")
                nc.vector.reciprocal(out=r2[:], in_=s2[:])
                r2b = A(r2, [[NBLK, 120], [1, NBLK], [0, N]])
                nc.vector.tensor_tensor(out=A(ef_all, [[EW, 120], [N, NBLK], [1, N]]),
                                        in0=em2v, in1=r2b, op=OP.mult)

                # ---------- phase 5: d2p ----------
                for kk in range(NBLK):
                    base = 120 * kk
                    efT = wp.tile([N, 120], F32, tag="efT")
                    pt = PM.tile([128, 512], F32, tag="med")
                    nc.tensor.transpose(pt[:N, :120],
                                        ef_all[:, N * kk:N * (kk + 1)], ident[:120, :120])
                    nc.vector.tensor_copy(efT[:], pt[:N, :120])
                    pnat = wp.tile([N, 4 * D], F32, tag="pnat")
                    pt2 = PM.tile([128, 512], F32, tag="med")
                    for b in range(4):
                        nc.tensor.transpose(pt2[:N, 128 * b:128 * (b + 1)],
                                            vc[:, base + 30 * b:base + 30 * b + N],
                                            ident[:])
                    nc.vector.tensor_copy(pnat[:], pt2[:N, :])
                    pag = PM.tile([128, 512], F32, tag="med")
                    for b in range(4):
                        nc.tensor.matmul(pag[:, 30 * b:30 * b + N],
                                         _mm(pnat[:, 128 * b:128 * (b + 1)]),
                                         _mm(efT[:, 30 * b:30 * b + N]),
                                         start=True, stop=True)
                    aggr = wp.tile([128, 120], F32, tag="aggr")
                    nc.vector.tensor_copy(aggr[:], pag[:, :120])
                    hdp = [wp.tile([128, 120], F32, tag=f"hdp{h}", name=f"hdp{h}") for h in range(2)]
                    for h in range(2):
                        pm_ = PM.tile([128, 512], F32, tag="med")
                        nc.tensor.matmul(pm_[:, :120],
                                         _mm(w["dpw1T"][0][:, 128 * h:128 * (h + 1)]),
                                         _mm(vc[:, base:base + 120]),
                                         start=True, stop=False)
                        nc.tensor.matmul(pm_[:, :120],
                                         _mm(w["dpw1T"][1][:, 128 * h:128 * (h + 1)]),
                                         _mm(aggr[:]), start=False, stop=True)
                        act_lrelu(hdp[h][:], pm_[:, :120],
                                  w["dpg1"][:, h:h + 1], w["dpb1"][:, h:h + 1])
                    pm_ = PM.tile([128, 512], F32, tag="med")
                    for k in range(2):
                        nc.tensor.matmul(pm_[:, :120], _mm(w["dpw2T"][k][:]),
                                         _mm(hdp[k][:]), start=(k == 0), stop=(k == 1))
                    act_lrelu(vn[:, base:base + 120], pm_[:, :120],
                              w["dpg2"][:], w["dpb2"][:])

    nc.compile()
    return nc


def _get_nc():
    key = MM_MODE
    if key not in _NC_CACHE:
        _NC_CACHE[key] = build_nc()
    return _NC_CACHE[key]


# ---------------------------------------------------------------------------
# Cached PJRT runner.  run_bass_kernel_spmd builds a fresh jax.jit closure on
# every call (full retrace + XLA/PJRT compile + 8x replicated weight upload
# per invocation).  Here the jitted shard_map executable is built once and
# the replicated weights live on-device across calls.
# ---------------------------------------------------------------------------
_RUNNER_CACHE = {}
_DATA_NAMES = ("point_node", "point_edge", "distribution_node", "distribution_edge")


def _get_runner():
    key = MM_MODE
    if key in _RUNNER_CACHE:
        return _RUNNER_CACHE[key]
    import jax
    from jax.experimental.shard_map import shard_map
    from jax.sharding import Mesh, NamedSharding, PartitionSpec as P
    from concourse import bass2jax

    nc = _get_nc()
    bass2jax.install_neuronx_cc_hook()
    assert nc.dbg_addr is None, "rebuild with debug=False"
    partition_name = nc.partition_id_tensor.name if nc.partition_id_tensor else None

    in_names, out_names, out_avals = [], [], []
    for alloc in nc.m.functions[0].allocations:
        if not isinstance(alloc, mybir.MemoryLocationSet):
            continue
        name = alloc.memorylocations[0].name
        if alloc.kind == "ExternalInput":
            if name != partition_name:
                in_names.append(name)
        elif alloc.kind == "ExternalOutput":
            out_names.append(name)
            out_avals.append(jax.core.ShapedArray(
                tuple(alloc.tensor_shape), mybir.dt.np(alloc.dtype)))
    # The kernel writes every element of "out", so no pre-zeroed donated
    # output buffers are needed — PJRT-allocated (uninit) results are fine.
    all_names = tuple(in_names + ([partition_name] if partition_name else []))

    devices = jax.devices()[:NCORES]
    mesh = Mesh(np.asarray(devices), ("core",))
    repl_sh = NamedSharding(mesh, P())

    def _body(*args):
        operands = list(args)
        if partition_name is not None:
            operands.append(bass2jax.partition_id_tensor())
        return tuple(bass2jax._bass_exec_p.bind(
            *operands,
            out_avals=tuple(out_avals),
            in_names=all_names,
            out_names=tuple(out_names),
            lowering_input_output_aliases=(),
            sim_require_finite=True,
            sim_require_nnan=True,
            nc=nc,
        ))

    in_specs = tuple(P("core") if nm in _DATA_NAMES else P() for nm in in_names)
    out_specs = (P("core"),) * len(out_names)
    fn = jax.jit(
        shard_map(_body, mesh=mesh, in_specs=in_specs, out_specs=out_specs,
                  check_rep=False),
        keep_unused=True)

    runner = dict(fn=fn, devices=devices, in_names=in_names,
                  out_names=out_names, repl_sh=repl_sh, jax=jax, wcache={})
    _RUNNER_CACHE[key] = runner
    return runner


def _hash_arr(arr):
    import hashlib
    a = arr if arr.flags["C_CONTIGUOUS"] else np.ascontiguousarray(arr)
    return hashlib.blake2b(a.data, digest_size=16).digest()


def _kernel_sharded(r, inputs):
    jax, wcache = r["jax"], r["wcache"]
    from concurrent.futures import ThreadPoolExecutor

    pool = r.setdefault("pool", ThreadPoolExecutor(4))
    conv = {name: pool.submit(
        lambda nm=name: np.ascontiguousarray(np.asarray(inputs[nm], dtype=IO_NP)))
        for name in _DATA_NAMES}
    args = []
    for name in r["in_names"]:
        if name in _DATA_NAMES:
            args.append(conv[name].result())
        else:
            arr = np.ascontiguousarray(np.asarray(inputs[name], dtype=np.float32))
            h = _hash_arr(arr)
            cached = wcache.get(name)
            if cached is None or cached[0] != h:
                cached = (h, jax.device_put(arr, r["repl_sh"]))
                wcache[name] = cached
            args.append(cached[1])
    outs = r["fn"](*args)

    # g0 node_l2 = -|vi-vj|^2 of the RAW point_node input — computed on the
    # host (f64 gram to dodge cancellation) while the device round trip is
    # in flight, so it ships zero bytes.
    v = np.asarray(inputs["point_node"], np.float64)
    gram = np.matmul(v, v.transpose(0, 2, 1))
    n2 = np.einsum("bnd,bnd->bn", v, v)
    g0l2 = (2.0 * gram - n2[:, :, None]) - n2[:, None, :]
    idx = np.arange(N)
    g0l2[:, idx, idx] = 0.0

    res = np.empty((G, 3, B, N, N), np.float32)
    res[0, 1] = g0l2
    # device channels: 0=g0 pe, 1=g0 de, 2=g1 pe, 3=g1 l2, 4=g1 de
    chmap = ((0, 0), (0, 2), (1, 0), (1, 1), (1, 2))
    try:
        shards = sorted(outs[0].addressable_shards,
                        key=lambda s: s.index[0].start or 0)
        assert len(shards) == NCORES
        for s in shards:
            s.data.copy_to_host_async()
        percore = [np.asarray(s.data) for s in shards]
    except Exception:
        o = np.asarray(outs[0]).reshape(NCORES, 5, BC, N, N)
        percore = [o[c] for c in range(NCORES)]
    for c, a in enumerate(percore):
        sl = slice(c * BC, (c + 1) * BC)
        for ch, (gg, cc) in enumerate(chmap):
            res[gg, cc, sl] = a[ch]
    return res


def kernel(**inputs):
    return _kernel_sharded(_get_runner(), inputs)



# revision 34
# speedup vs baseline: 1.0179x; 1.0179x over previous
"""DPGN (gnn_message_passing) fused Trainium2 kernel.

Sharding: pure data parallel over meta-batch B=256 -> 8 cores x 32 samples.
Per core, samples run in 8 blocks of 4. The whole 2-generation DPGN step is
fused on-chip; only inputs/outputs touch HBM.

Layouts (per block of 4 samples b=0..3):
  vT         [128, 120]  point features: channel on partition, (b,i) on free
  d2         [128, 3600] pairwise sq-dists: (b,i,j) on free
  edge tiles [128, 240]  rows 32b+i (32-aligned), free (kk,j)
  dist feats [128, *]    row-group packed: rows 32b+c (c<25)

Runtime: the axon PJRT tunnel has a ~68ms fixed RTT and ~100MB/s streams,
which dominates wall time (HW exec is <8ms).  Hence:
  - the jitted shard_map executable is built ONCE and cached (the stock
    run_bass_kernel_spmd retraces+recompiles per call, ~500ms overhead);
  - replicated weights are content-hashed and live on-device across calls;
  - data inputs and outputs cross the wire as fp16 (tolerance gate 2e-2,
    fp16 wire costs ~1e-3), converted on-chip / on-host;
  - no donated zero output buffers (every output element is written);
  - g0 node_l2 (= -|vi-vj|^2 of the raw point_node input) is recomputed
    on the host in f64 while the device round trip is in flight, so the
    device ships 5 output channels instead of 6.
"""
import sys
sys.path.insert(0, "/opt/trn_rl_repo")
from contextlib import ExitStack

import numpy as np
import concourse.bass as bass
import concourse.bacc as bacc
import concourse.tile as tile
from concourse import mybir
from concourse.bass_utils import run_bass_kernel_spmd
from concourse.masks import make_identity

F32 = mybir.dt.float32
AF = mybir.ActivationFunctionType
OP = mybir.AluOpType
AX = mybir.AxisListType

G, B, N, S, D = 2, 256, 30, 25, 128
NCORES = 8
BC = B // NCORES          # 32 samples per core
NBLK = BC // 4            # 8 blocks of 4 samples
EW = NBLK * N             # 240
NEG = 0.01
BN_SCALE = float(1.0 / np.sqrt(1.0 + 1e-5))
EPS_L1 = 1e-12

# matmul operand mode: "f32" (exact, 4 cyc/row) | "f32r" (reduced-precision mul, 1 cyc/row)
MM_MODE = "f32"
# leaky-relu implementation: "act" (1 ScalarE op; not in CoreSim) | "dve" (Identity + DVE max)
LRELU_ON = "act"
# debug: comma set of enabled parts: "setup,p1,p2,p3,p4,p5" (default all)
import os as _os
PHASES = set((_os.environ.get("KPHASES") or "setup,p1,p2,p3,p4,p5").split(","))
KGENS = int(_os.environ.get("KGENS") or G)
KREPEAT = int(_os.environ.get("KREPEAT") or 1)

_NC_CACHE = {}


BF16 = mybir.dt.bfloat16
F32R = mybir.dt.float32r
F16 = mybir.dt.float16
# wire dtype for the 4 data inputs and the output (tunnel-bandwidth bound;
# tolerance gate is 2e-2, fp16 wire adds ~1e-3)
IO_DT = F16 if (_os.environ.get("KIODT") or "f16") == "f16" else F32
IO_NP = np.float16 if IO_DT == F16 else np.float32


def _dt_point():   # d2, h1, w1T, w2T (base-0 matmuls only)
    if MM_MODE == "hybrid":
        return F32R
    if MM_MODE == "bf16":
        return BF16
    return F32


def _dt_flex():    # dist chain (col/row-tiled matmuls)
    if MM_MODE in ("hybrid", "bf16"):
        return BF16
    return F32


def _dt_s():       # h2 / w3T (s-path: accuracy-sensitive)
    return BF16 if MM_MODE == "bf16" else F32


def _mm(ap):
    return ap


def A(t, ap, off=0):
    return bass.AP(tensor=t.tensor, offset=t.offset + off, ap=ap)


def build_nc():
    nc = bacc.Bacc("TRN2", target_bir_lowering=False, debug=False)
    MDP = _dt_point()
    MDF = _dt_flex()
    MDS = _dt_s()

    pn_d = nc.dram_tensor("point_node", [BC, N, D], IO_DT, kind="ExternalInput")
    pe_d = nc.dram_tensor("point_edge", [BC, N, N], IO_DT, kind="ExternalInput")
    dn_d = nc.dram_tensor("distribution_node", [BC, N, S], IO_DT, kind="ExternalInput")
    de_d = nc.dram_tensor("distribution_edge", [BC, N, N], IO_DT, kind="ExternalInput")
    wd = {}
    for name, shape in [
        ("ps_w1", [G, 2 * D, D]), ("ps_g1", [G, 2 * D]), ("ps_b1", [G, 2 * D]),
        ("ps_w2", [G, D, 2 * D]), ("ps_g2", [G, D]), ("ps_b2", [G, D]),
        ("ps_w3", [G, 1, D]), ("ps_b3", [G, 1]),
        ("p2d_w", [G, S, 2 * S]), ("p2d_b", [G, S]),
        ("ds_w1", [G, 2 * S, S]), ("ds_g1", [G, 2 * S]), ("ds_b1", [G, 2 * S]),
        ("ds_w2", [G, S, 2 * S]), ("ds_g2", [G, S]), ("ds_b2", [G, S]),
        ("ds_w3", [G, 1, S]), ("ds_b3", [G, 1]),
        ("dp_w1", [G, 2 * D, 2 * D]), ("dp_g1", [G, 2 * D]), ("dp_b1", [G, 2 * D]),
        ("dp_w2", [G, D, 2 * D]), ("dp_g2", [G, D]), ("dp_b2", [G, D]),
    ]:
        wd[name] = nc.dram_tensor(name, shape, F32, kind="ExternalInput")
    # output channels: 0=g0 point_edge, 1=g0 dist_edge, 2=g1 point_edge,
    # 3=g1 node_l2, 4=g1 dist_edge.  g0 node_l2 is recomputed on the host
    # from the raw point_node input (cheaper than shipping it).
    out_d = nc.dram_tensor("out", [5, BC, N, N], IO_DT, kind="ExternalOutput")
    OCH, OB = BC * N * N, N * N

    with tile.TileContext(nc) as tc, ExitStack() as ctx:
        cp = ctx.enter_context(tc.tile_pool(name="cpool", bufs=1))
        vp = ctx.enter_context(tc.tile_pool(name="vpool", bufs=1))
        wp = ctx.enter_context(tc.tile_pool(name="wpool", bufs=2))
        ep = ctx.enter_context(tc.tile_pool(name="epool", bufs=2))
        PB = ctx.enter_context(tc.tile_pool(name="PB", bufs=2, space="PSUM"))
        PM = ctx.enter_context(tc.tile_pool(name="PM", bufs=3, space="PSUM"))

        # ================= constants =================
        ident = cp.tile([128, 128], F32, tag="ident")
        make_identity(nc, ident[:])
        off_m = cp.tile([120, N], F32, tag="off_m")           # 1 - eye (30-stride)
        eyeeps = cp.tile([120, N], F32, tag="eyeeps")         # eye + 1e-6
        nc.gpsimd.memset(off_m[:], 1.0)
        nc.gpsimd.memset(eyeeps[:], 1e-6)
        for t, fill in ((off_m, 0.0), (eyeeps, 1.0 + 1e-6)):
            nc.gpsimd.affine_select(
                out=t[0:N, :], in_=t[0:N, :],
                compare_op=OP.not_equal, fill=fill, base=0,
                pattern=[[-1, N]], channel_multiplier=1)
            for b in range(1, 4):
                nc.sync.dma_start(out=t[30 * b:30 * b + N, :], in_=t[0:N, :])
        Eb = cp.tile([S, 4, 128], F32, tag="Eb")              # 1 at (c, 32b+c)
        nc.gpsimd.memset(Eb[:], 0.0)
        for b in range(4):
            nc.gpsimd.affine_select(
                out=Eb[:, b, :], in_=Eb[:, b, :], compare_op=OP.not_equal,
                fill=1.0, base=32 * b, pattern=[[-1, 128]], channel_multiplier=1)
        E2 = cp.tile([2 * S, 2, 128], F32, tag="E2")          # 1 at (c, 64q+c)
        nc.gpsimd.memset(E2[:], 0.0)
        for q in range(2):
            nc.gpsimd.affine_select(
                out=E2[:, q, :], in_=E2[:, q, :], compare_op=OP.not_equal,
                fill=1.0, base=64 * q, pattern=[[-1, 128]], channel_multiplier=1)
        onesT = cp.tile([128, 32], F32, tag="onesT")
        ones_f = cp.tile([128, 32], F32, tag="ones_f")
        nc.vector.memset(ones_f[:], 0.0)
        nc.vector.memset(ones_f[:, 0:1], 1.0)
        nc.vector.tensor_copy(onesT[:], ones_f[:])


        def act_lrelu(out_ap, in_ap, scale, bias):
            if LRELU_ON == "act":
                # Prelu == leaky relu; lives in the sigmoid table set (Lrelu does not,
                # and mixing Lrelu+Sigmoid table loads crashes the ACT engine)
                nc.scalar.activation(out=out_ap, in_=in_ap, func=AF.Prelu,
                                     alpha=NEG, scale=scale, bias=bias)
            elif LRELU_ON == "actsim":
                # timing-equivalent stand-in for CoreSim (values wrong: no lrelu)
                nc.scalar.activation(out=out_ap, in_=in_ap, func=AF.Identity,
                                     scale=scale, bias=bias)
            else:
                nc.scalar.activation(out=out_ap, in_=in_ap, func=AF.Identity,
                                     scale=scale, bias=bias)
                nc.vector.scalar_tensor_tensor(out=out_ap, in0=out_ap, scalar=NEG,
                                               in1=out_ap, op0=OP.mult, op1=OP.max)

        def load_col(name, g, n, tag, blocks=1, scale=None):
            t = cp.tile([128, blocks], F32, tag=tag)
            if blocks > 1:
                src = bass.AP(tensor=wd[name], offset=g * n * blocks,
                              ap=[[1, n], [n, blocks]])
                dst = A(t, [[blocks, n], [1, blocks]])
            else:
                src = bass.AP(tensor=wd[name], offset=g * n, ap=[[1, n]])
                dst = A(t, [[1, n], [1, 1]])
            nc.sync.dma_start(out=dst, in_=src)
            if scale is not None:
                nc.vector.tensor_scalar(out=t[:n, :], in0=t[:n, :], scalar1=scale,
                                        scalar2=None, op0=OP.mult)
            return t

        def load_col_rep(name, g, n, tag, bases, scale=None):
            t = cp.tile([128, 1], F32, tag=tag)
            nc.vector.memset(t[:], 0.0)
            src = bass.AP(tensor=wd[name], offset=g * n, ap=[[1, n], [1, 1]])
            for bb in bases:
                nc.sync.dma_start(out=t[bb:bb + n, :], in_=src)
            if scale is not None:
                for bb in bases:
                    nc.vector.tensor_scalar(out=t[bb:bb + n, :], in0=t[bb:bb + n, :],
                                            scalar1=scale, scalar2=None, op0=OP.mult)
            return t

        def transpose_to(dst_ap, src_ap, idn):
            p = src_ap.partition_size()
            f = src_ap.free_size()
            pt = PM.tile([128, 512], F32, tag="med")
            nc.tensor.transpose(pt[:f, :p], src_ap, idn)
            nc.vector.tensor_copy(dst_ap, pt[:f, :p])

        # ================= weights =================
        W = {g: {} for g in range(G)}
        for g in range(G):
            w = W[g]
            w1T = cp.tile([128, 2 * D], MDP, tag=f"w1T{g}")
            for h in range(2):
                tmp = wp.tile([128, D], F32, tag="wload")
                nc.sync.dma_start(out=tmp[:], in_=wd["ps_w1"][g, 128 * h:128 * (h + 1), :])
                transpose_to(w1T[:, 128 * h:128 * (h + 1)], tmp[:], ident[:])
            w["w1T"] = w1T
            w2T = cp.tile([128, 2, D], MDP, tag=f"w2T{g}")
            tmp = wp.tile([128, 2 * D], F32, tag="wload2")
            nc.sync.dma_start(out=tmp[:], in_=wd["ps_w2"][g])
            for k in range(2):
                transpose_to(w2T[:, k, :], tmp[:, 128 * k:128 * (k + 1)], ident[:])
            w["w2T"] = w2T
            w3T = cp.tile([128, 32], MDS, tag=f"w3T{g}")
            w3f = wp.tile([128, 32], F32, tag="wst")
            nc.vector.memset(w3f[:], 0.0)
            nc.sync.dma_start(out=A(w3f, [[32, 128], [1, 1]]),
                              in_=bass.AP(tensor=wd["ps_w3"], offset=g * D, ap=[[1, D]]))
            nc.vector.tensor_copy(w3T[:], w3f[:])
            w["w3T"] = w3T
            w["gs1"] = load_col("ps_g1", g, 128, f"gs1{g}", 2, scale=BN_SCALE)
            w["bs1"] = load_col("ps_b1", g, 128, f"bs1{g}", 2)
            w["gs2"] = load_col("ps_g2", g, 128, f"gs2{g}", scale=BN_SCALE)
            w["bs2"] = load_col("ps_b2", g, 128, f"bs2{g}")
            b3bc = cp.tile([128, 1], F32, tag=f"b3bc{g}")
            nc.sync.dma_start(out=b3bc[:],
                              in_=bass.AP(tensor=wd["ps_b3"], offset=g, ap=[[0, 128], [1, 1]]))
            w["b3bc"] = b3bc

            tmp = wp.tile([S, 2 * S], F32, tag="wload3")
            nc.sync.dma_start(out=tmp[:], in_=wd["p2d_w"][g])
            p2dA = cp.tile([S, 32], F32, tag=f"p2dA{g}")
            nc.vector.memset(p2dA[:], 0.0)
            transpose_to(p2dA[:, 0:S], tmp[:, 0:S], ident[:S, :S])
            p2dAr = cp.tile([128, 32], F32, tag=f"p2dAr{g}")
            nc.vector.memset(p2dAr[:], 0.0)
            ptA = PM.tile([128, 512], F32, tag="med")
            for b in range(4):
                nc.tensor.matmul(ptA[:, :32], Eb[:, b, :], p2dA[:],
                                 start=(b == 0), stop=(b == 3))
            nc.vector.tensor_copy(p2dAr[:, :], ptA[:, :32])
            w["p2dAr"] = p2dAr
            p2dBf = wp.tile([S, S], F32, tag="p2dBf")
            transpose_to(p2dBf[:], tmp[:, S:2 * S], ident[:S, :S])
            p2dB = cp.tile([128, 32], F32, tag=f"p2dB{g}")
            nc.vector.memset(p2dB[:], 0.0)
            pt = PM.tile([128, 512], F32, tag="med")
            for b in range(4):
                nc.tensor.matmul(pt[:, :S], Eb[:, b, :], p2dBf[:],
                                 start=(b == 0), stop=(b == 3))
            nc.vector.tensor_copy(p2dB[:, 0:S], pt[:, :S])
            w["p2dA"], w["p2dB"] = p2dA, p2dB
            w["p2db"] = load_col_rep("p2d_b", g, S, f"p2db{g}", [0, 32, 64, 96])

            tmp = wp.tile([2 * S, S], F32, tag="wload4")
            nc.sync.dma_start(out=tmp[:], in_=wd["ds_w1"][g])
            dsw1f = wp.tile([S, 2 * S], F32, tag="dsw1f")
            transpose_to(dsw1f[:], tmp[:], ident[:2 * S, :2 * S])
            dsw1 = cp.tile([128, 64], MDF, tag=f"dsw1{g}")
            d1f = wp.tile([128, 64], F32, tag="wst2")
            nc.vector.memset(d1f[:], 0.0)
            pt = PM.tile([128, 512], F32, tag="med")
            for b in range(4):
                nc.tensor.matmul(pt[:, :2 * S], Eb[:, b, :], dsw1f[:],
                                 start=(b == 0), stop=(b == 3))
            nc.vector.tensor_copy(d1f[:, 0:2 * S], pt[:, :2 * S])
            nc.vector.tensor_copy(dsw1[:], d1f[:])
            w["dsw1"] = dsw1
            tmp = wp.tile([S, 2 * S], F32, tag="wload5")
            nc.sync.dma_start(out=tmp[:], in_=wd["ds_w2"][g])
            dsw2f = wp.tile([2 * S, S], F32, tag="dsw2f")
            transpose_to(dsw2f[:], tmp[:], ident[:S, :S])
            dsw2 = cp.tile([128, 32], MDF, tag=f"dsw2{g}")
            d2f = wp.tile([128, 32], F32, tag="wst3")
            nc.vector.memset(d2f[:], 0.0)
            pt = PM.tile([128, 512], F32, tag="med")
            for q in range(2):
                nc.tensor.matmul(pt[:, :S], E2[:, q, :], dsw2f[:],
                                 start=(q == 0), stop=(q == 1))
            nc.vector.tensor_copy(d2f[:, 0:S], pt[:, :S])
            nc.vector.tensor_copy(dsw2[:], d2f[:])
            w["dsw2"] = dsw2
            dsw3 = cp.tile([128, 32], MDF, tag=f"dsw3{g}")
            d3f = wp.tile([128, 32], F32, tag="wst4")
            nc.vector.memset(d3f[:], 0.0)
            for b in range(4):
                nc.sync.dma_start(out=d3f[32 * b:32 * b + S, 0:1],
                                  in_=bass.AP(tensor=wd["ds_w3"], offset=g * S, ap=[[1, S], [1, 1]]))
            nc.vector.tensor_copy(dsw3[:], d3f[:])
            w["dsw3"] = dsw3
            w["dsg1"] = load_col_rep("ds_g1", g, 2 * S, f"dsg1{g}", [0, 64], scale=BN_SCALE)
            w["dsb1"] = load_col_rep("ds_b1", g, 2 * S, f"dsb1{g}", [0, 64])
            w["dsg2"] = load_col_rep("ds_g2", g, S, f"dsg2{g}", [0, 32, 64, 96], scale=BN_SCALE)
            w["dsb2"] = load_col_rep("ds_b2", g, S, f"dsb2{g}", [0, 32, 64, 96])
            dsb3bc = cp.tile([128, 1], F32, tag=f"dsb3bc{g}")
            nc.sync.dma_start(out=dsb3bc[:],
                              in_=bass.AP(tensor=wd["ds_b3"], offset=g, ap=[[0, 128], [1, 1]]))
            w["dsb3bc"] = dsb3bc

            if g < G - 1:
                dpw1T = [cp.tile([128, 2 * D], F32, tag=f"dpw1T{g}_{k}", name=f"dpw1T{g}_{k}") for k in range(2)]
                for r in range(2):
                    tmp = wp.tile([128, 2 * D], F32, tag="wload6")
                    nc.sync.dma_start(out=tmp[:], in_=wd["dp_w1"][g, 128 * r:128 * (r + 1), :])
                    for k in range(2):
                        transpose_to(dpw1T[k][:, 128 * r:128 * (r + 1)],
                                     tmp[:, 128 * k:128 * (k + 1)], ident[:])
                w["dpw1T"] = dpw1T
                tmp = wp.tile([128, 2 * D], F32, tag="wload7")
                nc.sync.dma_start(out=tmp[:], in_=wd["dp_w2"][g])
                dpw2T = [cp.tile([128, D], F32, tag=f"dpw2T{g}_{k}", name=f"dpw2T{g}_{k}") for k in range(2)]
                for k in range(2):
                    transpose_to(dpw2T[k][:], tmp[:, 128 * k:128 * (k + 1)], ident[:])
                w["dpw2T"] = dpw2T
                w["dpg1"] = load_col("dp_g1", g, 128, f"dpg1{g}", 2, scale=BN_SCALE)
                w["dpb1"] = load_col("dp_b1", g, 128, f"dpb1{g}", 2)
                w["dpg2"] = load_col("dp_g2", g, 128, f"dpg2{g}", scale=BN_SCALE)
                w["dpb2"] = load_col("dp_b2", g, 128, f"dpb2{g}")

        # ================= persistent state =================
        vT = [vp.tile([128, BC * N], F32, tag=f"vT{i}", name=f"vT{i}") for i in range(2)]
        dn_rg = vp.tile([128, EW], F32, tag="dn_rg")
        pe_all = vp.tile([120, EW], F32, tag="pe_all")
        de_all = vp.tile([120, EW], F32, tag="de_all")
        s_all = vp.tile([120, EW], F32, tag="s_all")
        sds_all = vp.tile([120, EW], F32, tag="sds_all")
        ef_all = vp.tile([120, EW], F32, tag="ef_all")
        for t in (pe_all, de_all, s_all, sds_all, ef_all, dn_rg, vT[0], vT[1]):
            nc.gpsimd.memset(t[:], 0.0)

        # ---- gen-1 input staging ----
        for kk in range(NBLK):
            pf = wp.tile([120, D], F32, tag="pnflat")
            if IO_DT == F32:
                nc.sync.dma_start(out=pf[:], in_=pn_d[4 * kk:4 * (kk + 1)].rearrange("b n d -> (b n) d"))
            else:
                pf16 = wp.tile([120, D], IO_DT, tag="pnflat16")
                nc.sync.dma_start(out=pf16[:], in_=pn_d[4 * kk:4 * (kk + 1)].rearrange("b n d -> (b n) d"))
                nc.vector.tensor_copy(pf[:], pf16[:])
            pt = PM.tile([128, 512], F32, tag="med")
            nc.tensor.transpose(pt[:, :120], pf[:], ident[:120, :120])
            nc.vector.tensor_copy(vT[0][:, 120 * kk:120 * (kk + 1)], pt[:, :120])

            df = wp.tile([120, S], F32, tag="dnflat")
            if IO_DT == F32:
                nc.sync.dma_start(out=df[:], in_=dn_d[4 * kk:4 * (kk + 1)].rearrange("b n s -> (b n) s"))
            else:
                df16 = wp.tile([120, S], IO_DT, tag="dnflat16")
                nc.sync.dma_start(out=df16[:], in_=dn_d[4 * kk:4 * (kk + 1)].rearrange("b n s -> (b n) s"))
                nc.vector.tensor_copy(df[:], df16[:])
            pt2 = PM.tile([128, 512], F32, tag="med")
            nc.tensor.transpose(pt2[:S, :120], df[:], ident[:120, :120])
            dnf = wp.tile([S, 120], F32, tag="dnf")
            nc.vector.tensor_copy(dnf[:], pt2[:S, :120])
            pt3 = PM.tile([128, 512], F32, tag="med")
            for b in range(4):
                nc.tensor.matmul(pt3[:, :N], Eb[:, b, :], dnf[:, 30 * b:30 * b + N],
                                 start=(b == 0), stop=(b == 3))
            nc.vector.tensor_copy(dn_rg[:, N * kk:N * (kk + 1)], pt3[:, :N])

            for (ed, et) in ((pe_d, pe_all), (de_d, de_all)):
                if IO_DT == F32:
                    nc.sync.dma_start(out=et[:, N * kk:N * (kk + 1)],
                                      in_=ed[4 * kk:4 * (kk + 1)].rearrange("b n m -> (b n) m"))
                else:
                    e16 = wp.tile([120, N], IO_DT, tag="edge16")
                    nc.sync.dma_start(out=e16[:],
                                      in_=ed[4 * kk:4 * (kk + 1)].rearrange("b n m -> (b n) m"))
                    nc.vector.tensor_copy(et[:, N * kk:N * (kk + 1)], e16[:])

        def edge_update(g, w, e_all, sig_src, b3bc, out_ch):
            ssig = ep.tile([120, EW], F32, tag="ssig")
            nc.scalar.activation(out=ssig[:], in_=sig_src[:], func=AF.Sigmoid,
                                 bias=b3bc[:120, :], scale=1.0)
            em = ep.tile([120, EW], F32, tag="em")
            offb = A(off_m, [[N, 120], [0, NBLK], [1, N]])
            emv = A(em, [[EW, 120], [N, NBLK], [1, N]])
            nc.vector.tensor_tensor(out=emv, in0=A(e_all, [[EW, 120], [N, NBLK], [1, N]]),
                                    in1=offb, op=OP.mult)
            esum = ep.tile([120, NBLK], F32, tag="esum")
            nc.vector.tensor_reduce(out=esum[:], in_=emv, axis=AX.X, op=OP.add)
            t = ep.tile([120, EW], F32, tag="t")
            nc.vector.tensor_tensor(out=t[:], in0=ssig[:], in1=em[:], op=OP.mult)
            ts = ep.tile([120, NBLK], F32, tag="ts")
            nc.vector.tensor_reduce(out=ts[:], in_=A(t, [[EW, 120], [N, NBLK], [1, N]]),
                                    axis=AX.X, op=OP.add)
            nc.vector.tensor_scalar(out=ts[:], in0=ts[:], scalar1=EPS_L1,
                                    scalar2=None, op0=OP.max)
            r = ep.tile([120, NBLK], F32, tag="r")
            nc.vector.reciprocal(out=r[:], in_=ts[:])
            nc.vector.tensor_tensor(out=r[:], in0=r[:], in1=esum[:], op=OP.mult)
            e2 = ep.tile([120, EW], F32, tag="e2")
            rb = A(r, [[NBLK, 120], [1, NBLK], [0, N]])
            e2v = A(e2, [[EW, 120], [N, NBLK], [1, N]])
            nc.vector.tensor_tensor(out=e2v, in0=A(t, [[EW, 120], [N, NBLK], [1, N]]),
                                    in1=rb, op=OP.mult)
            eyb = A(eyeeps, [[N, 120], [0, NBLK], [1, N]])
            nc.vector.tensor_tensor(out=e2v, in0=e2v, in1=eyb, op=OP.add)
            rsum = ep.tile([120, NBLK], F32, tag="rsum")
            nc.vector.tensor_reduce(out=rsum[:], in_=e2v, axis=AX.X, op=OP.add)
            rr = ep.tile([120, NBLK], F32, tag="rr")
            nc.vector.reciprocal(out=rr[:], in_=rsum[:])
            rrb = A(rr, [[NBLK, 120], [1, NBLK], [0, N]])
            nc.vector.tensor_tensor(out=A(e_all, [[EW, 120], [N, NBLK], [1, N]]),
                                    in0=e2v, in1=rrb, op=OP.mult)
            if IO_DT == F32:
                esrc = e_all
            else:
                esrc = ep.tile([120, EW], IO_DT, tag="eo16")
                nc.vector.tensor_copy(esrc[:], e_all[:])
            for kk in range(NBLK):
                dst = bass.AP(tensor=out_d,
                              offset=out_ch * OCH + 4 * kk * OB,
                              ap=[[N, 120], [1, N]])
                nc.sync.dma_start(out=dst, in_=esrc[:, N * kk:N * (kk + 1)])

        PSUM_PAT = [[1024, 128], [512, 2], [1, 450]]

        # ================= generations =================
        for _rep in range(KREPEAT):
         for g in range(KGENS):
            w = W[g]
            vc, vn = vT[g % 2], vT[(g + 1) % 2]

            # ---------- phase 1: point sim MLP ----------
            for kk in range(NBLK if "p1" in PHASES else 0):
                base = 120 * kk
                d2 = wp.tile([128, 4 * N * N], MDP, tag="d2")
                vi = A(vc, [[BC * N, 128], [N, 4], [1, N], [0, N]], off=base)
                vj = A(vc, [[BC * N, 128], [N, 4], [0, N], [1, N]], off=base)
                dv = A(d2, [[3600, 128], [900, 4], [N, N], [1, N]])
                nc.vector.tensor_tensor(out=dv, in0=vi, in1=vj, op=OP.subtract)
                nc.vector.tensor_tensor(out=d2[:], in0=d2[:], in1=d2[:], op=OP.mult)
                h2 = wp.tile([128, 4 * N * N], MDS, tag="h2")
                for bb in range(4):   # per sample
                    h1 = [wp.tile([128, N * N], MDP, tag=f"h1_{h}", name=f"h1_{h}") for h in range(2)]
                    for h in range(2):
                        pb = PB.tile([128, 2, 512], F32, tag="big")
                        for p in range(2):
                            nc.tensor.matmul(pb[:, p, 0:450],
                                             _mm(w["w1T"][:, 128 * h:128 * (h + 1)]),
                                             _mm(d2[:, 900 * bb + 450 * p:900 * bb + 450 * (p + 1)]),
                                             start=True, stop=True)
                        act_lrelu(A(h1[h], [[900, 128], [450, 2], [1, 450]]),
                                  A(pb, PSUM_PAT),
                                  w["gs1"][:, h:h + 1], w["bs1"][:, h:h + 1])
                    pb = PB.tile([128, 2, 512], F32, tag="big")
                    for p in range(2):
                        for k in range(2):
                            nc.tensor.matmul(pb[:, p, 0:450],
                                             _mm(w["w2T"][:, k, :]),
                                             _mm(h1[k][:, 450 * p:450 * (p + 1)]),
                                             start=(k == 0), stop=(k == 1))
                    act_lrelu(A(h2, [[3600, 128], [450, 2], [1, 450]], off=900 * bb),
                              A(pb, PSUM_PAT), w["gs2"][:], w["bs2"][:])
                # s_pre and node_l2 via col-tiled M=1 matmuls
                # (g0 node_l2 is host-computed from raw point_node — skip stage 1)
                for stage in range(2 if g == 1 else 1):
                    rhs_t, lhs = (h2, w["w3T"]) if stage == 0 else (d2, onesT)
                    pb = PB.tile([128, 2, 512], F32, tag="big")
                    for p in range(2):
                        for b in range(4):
                            rr = rhs_t[:, 900 * b + 450 * p:900 * b + 450 * (p + 1)]
                            if stage == 1 and rr.dtype == F32R:
                                rr = rr.bitcast(F32)
                            nc.tensor.matmul(
                                pb[32 * b:32 * b + 32, p, 0:450],
                                lhs[:], rr,
                                start=True, stop=True, tile_position=(0, 32 * b))
                    stg = wp.tile([128, 900], F32, tag=f"stg{stage}")
                    if stage == 0:
                        nc.vector.tensor_copy(A(stg, [[900, 128], [450, 2], [1, 450]]),
                                              A(pb, PSUM_PAT))
                        src = A(stg, [[32 * 900, 4], [N, N], [1, N]])
                        nc.sync.dma_start(out=s_all[:, N * kk:N * (kk + 1)], in_=src)
                    else:
                        if IO_DT != F32:
                            stgo = wp.tile([128, 900], IO_DT, tag="stg1o", name="stgo")
                        else:
                            stgo = stg
                        nc.vector.tensor_scalar(
                            out=A(stgo, [[900, 128], [450, 2], [1, 450]]),
                            in0=A(pb, PSUM_PAT),
                            scalar1=-1.0, scalar2=None, op0=OP.mult)
                        for b in range(4):
                            src = A(stgo, [[900, 1], [N, N], [1, N]], off=32 * b * 900)
                            dst = bass.AP(tensor=out_d,
                                          offset=3 * OCH + (4 * kk + b) * OB,
                                          ap=[[N, N], [1, N]])
                            nc.sync.dma_start(out=dst, in_=src)

            # ---------- phase 2: point edge update ----------
            if "p2" in PHASES:
                edge_update(g, w, pe_all, s_all, w["b3bc"], 0 if g == 0 else 2)

            # ---------- phase 3: p2d + dist sim ----------
            for kk in range(NBLK if "p3" in PHASES else 0):
                peT = wp.tile([S, 120], F32, tag="peT")
                pt = PM.tile([128, 512], F32, tag="med")
                nc.tensor.transpose(pt[:S, :120], pe_all[:, N * kk:N * kk + S],
                                    ident[:120, :120])
                nc.vector.tensor_copy(peT[:], pt[:S, :120])
                ptg = PM.tile([128, 512], F32, tag="med")
                for b in range(4):
                    nc.tensor.matmul(ptg[:, :N], Eb[:, b, :],
                                     peT[:, 30 * b:30 * b + N],
                                     start=(b == 0), stop=(b == 3))
                peRG = wp.tile([128, N], F32, tag="peRG")
                nc.vector.tensor_copy(peRG[:], ptg[:, :N])
                pg = PM.tile([128, 512], F32, tag="med")
                for b in range(4):
                    nc.tensor.matmul(pg[32 * b:32 * b + 32, :N],
                                     _mm(w["p2dAr"][32 * b:32 * b + S, :]),
                                     _mm(peRG[32 * b:32 * b + S, :]),
                                     start=True, stop=False, tile_position=(32 * b, 32 * b))
                    nc.tensor.matmul(pg[32 * b:32 * b + 32, :N],
                                     _mm(w["p2dB"][32 * b:32 * b + S, :]),
                                     _mm(dn_rg[32 * b:32 * b + S, N * kk:N * (kk + 1)]),
                                     start=False, stop=True, tile_position=(32 * b, 32 * b))
                act_lrelu(dn_rg[:, N * kk:N * (kk + 1)], pg[:, :N], 1.0, w["p2db"][:])
                dd2 = wp.tile([128, N * N], MDF, tag="dd2")
                vi = A(dn_rg, [[EW, 128], [1, N], [0, N]], off=N * kk)
                vj = A(dn_rg, [[EW, 128], [0, N], [1, N]], off=N * kk)
                nc.vector.tensor_tensor(out=A(dd2, [[900, 128], [N, N], [1, N]]),
                                        in0=vi, in1=vj, op=OP.subtract)
                nc.vector.tensor_tensor(out=dd2[:], in0=dd2[:], in1=dd2[:], op=OP.mult)
                h1d = [wp.tile([128, N * N], MDF, tag=f"h1d{p}", name=f"h1d{p}") for p in range(2)]
                for pair in range(2):
                    pb = PB.tile([128, 2, 512], F32, tag="big")
                    for ck in range(2):
                        for q in range(2):
                            b = 2 * pair + q
                            nc.tensor.matmul(
                                pb[64 * q:64 * q + 64, ck, 0:450],
                                _mm(w["dsw1"][32 * b:32 * b + S, :]),
                                _mm(dd2[32 * b:32 * b + S, 450 * ck:450 * (ck + 1)]),
                                start=True, stop=True, tile_position=(32 * b, 64 * q))
                    act_lrelu(A(h1d[pair], [[900, 128], [450, 2], [1, 450]]),
                              A(pb, PSUM_PAT), w["dsg1"][:], w["dsb1"][:])
                h2d = wp.tile([128, N * N], MDF, tag="h2d")
                pb = PB.tile([128, 2, 512], F32, tag="big")
                for ck in range(2):
                    for pair in range(2):
                        for q in range(2):
                            b = 2 * pair + q
                            nc.tensor.matmul(
                                pb[32 * b:32 * b + 32, ck, 0:450],
                                _mm(w["dsw2"][64 * q:64 * q + 2 * S, :]),
                                _mm(h1d[pair][64 * q:64 * q + 2 * S, 450 * ck:450 * (ck + 1)]),
                                start=True, stop=True, tile_position=(64 * q, 32 * b))
                act_lrelu(A(h2d, [[900, 128], [450, 2], [1, 450]]),
                          A(pb, PSUM_PAT), w["dsg2"][:], w["dsb2"][:])
                pb = PB.tile([128, 2, 512], F32, tag="big")
                for ck in range(2):
                    for b in range(4):
                        nc.tensor.matmul(
                            pb[32 * b:32 * b + 32, ck, 0:450],
                            _mm(w["dsw3"][32 * b:32 * b + S, :]),
                            _mm(h2d[32 * b:32 * b + S, 450 * ck:450 * (ck + 1)]),
                            start=True, stop=True, tile_position=(32 * b, 32 * b))
                stg = wp.tile([128, 900], F32, tag="stgd")
                nc.vector.tensor_copy(A(stg, [[900, 128], [450, 2], [1, 450]]),
                                      A(pb, PSUM_PAT))
                src = A(stg, [[32 * 900, 4], [N, N], [1, N]])
                nc.sync.dma_start(out=sds_all[:, N * kk:N * (kk + 1)], in_=src)

            # ---------- phase 4: dist edge update (+ ef) ----------
            if "p4" in PHASES:
                edge_update(g, w, de_all, sds_all, w["dsb3bc"], 1 if g == 0 else 4)
            if g < G - 1 and "p5" in PHASES:
                em2 = ep.tile([120, EW], F32, tag="em2")
                offb = A(off_m, [[N, 120], [0, NBLK], [1, N]])
                em2v = A(em2, [[EW, 120], [N, NBLK], [1, N]])
                nc.vector.tensor_tensor(out=em2v,
                                        in0=A(de_all, [[EW, 120], [N, NBLK], [1, N]]),
                                        in1=offb, op=OP.mult)
                s2 = ep.tile([120, NBLK], F32, tag="s2")
                nc.vector.tensor_reduce(out=s2[:], in_=em2v, axis=AX.X, op=OP.add)
                nc.vector.tensor_scalar(out=s2[:], in0=s2[:], scalar1=EPS_L1,
                                        scalar2=None, op0=OP.max)
                r2 = ep.tile([120, NBLK], F32, tag="r2")
                nc.vector.reciprocal(out=r2[:], in_=s2[:])
                r2b = A(r2, [[NBLK, 120], [1, NBLK], [0, N]])
                nc.vector.tensor_tensor(out=A(ef_all, [[EW, 120], [N, NBLK], [1, N]]),
                                        in0=em2v, in1=r2b, op=OP.mult)

                # ---------- phase 5: d2p ----------
                for kk in range(NBLK):
                    base = 120 * kk
                    efT = wp.tile([N, 120], F32, tag="efT")
                    pt = PM.tile([128, 512], F32, tag="med")
                    nc.tensor.transpose(pt[:N, :120],
                                        ef_all[:, N * kk:N * (kk + 1)], ident[:120, :120])
                    nc.vector.tensor_copy(efT[:], pt[:N, :120])
                    pnat = wp.tile([N, 4 * D], F32, tag="pnat")
                    pt2 = PM.tile([128, 512], F32, tag="med")
                    for b in range(4):
                        nc.tensor.transpose(pt2[:N, 128 * b:128 * (b + 1)],
                                            vc[:, base + 30 * b:base + 30 * b + N],
                                            ident[:])
                    nc.vector.tensor_copy(pnat[:], pt2[:N, :])
                    pag = PM.tile([128, 512], F32, tag="med")
                    for b in range(4):
                        nc.tensor.matmul(pag[:, 30 * b:30 * b + N],
                                         _mm(pnat[:, 128 * b:128 * (b + 1)]),
                                         _mm(efT[:, 30 * b:30 * b + N]),
                                         start=True, stop=True)
                    aggr = wp.tile([128, 120], F32, tag="aggr")
                    nc.vector.tensor_copy(aggr[:], pag[:, :120])
                    hdp = [wp.tile([128, 120], F32, tag=f"hdp{h}", name=f"hdp{h}") for h in range(2)]
                    for h in range(2):
                        pm_ = PM.tile([128, 512], F32, tag="med")
                        nc.tensor.matmul(pm_[:, :120],
                                         _mm(w["dpw1T"][0][:, 128 * h:128 * (h + 1)]),
                                         _mm(vc[:, base:base + 120]),
                                         start=True, stop=False)
                        nc.tensor.matmul(pm_[:, :120],
                                         _mm(w["dpw1T"][1][:, 128 * h:128 * (h + 1)]),
                                         _mm(aggr[:]), start=False, stop=True)
                        act_lrelu(hdp[h][:], pm_[:, :120],
                                  w["dpg1"][:, h:h + 1], w["dpb1"][:, h:h + 1])
                    pm_ = PM.tile([128, 512], F32, tag="med")
                    for k in range(2):
                        nc.tensor.matmul(pm_[:, :120], _mm(w["dpw2T"][k][:]),
                                         _mm(hdp[k][:]), start=(k == 0), stop=(k == 1))
                    act_lrelu(vn[:, base:base + 120], pm_[:, :120],
                              w["dpg2"][:], w["dpb2"][:])

    nc.compile()
    return nc


def _get_nc():
    key = MM_MODE
    if key not in _NC_CACHE:
        _NC_CACHE[key] = build_nc()
    return _NC_CACHE[key]


# ---------------------------------------------------------------------------
# Cached PJRT runner.  run_bass_kernel_spmd builds a fresh jax.jit closure on
# every call (full retrace + XLA/PJRT compile + 8x replicated weight upload
# per invocation).  Here the jitted shard_map executable is built once and
# the replicated weights live on-device across calls.
# ---------------------------------------------------------------------------
_RUNNER_CACHE = {}
_DATA_NAMES = ("point_node", "point_edge", "distribution_node", "distribution_edge")


def _get_runner():
    key = MM_MODE
    if key in _RUNNER_CACHE:
        return _RUNNER_CACHE[key]
    import jax
    from jax.experimental.shard_map import shard_map
    from jax.sharding import Mesh, NamedSharding, PartitionSpec as P
    from concourse import bass2jax

    nc = _get_nc()
    bass2jax.install_neuronx_cc_hook()
    assert nc.dbg_addr is None, "rebuild with debug=False"
    partition_name = nc.partition_id_tensor.name if nc.partition_id_tensor else None

    in_names, out_names, out_avals = [], [], []
    for alloc in nc.m.functions[0].allocations:
        if not isinstance(alloc, mybir.MemoryLocationSet):
            continue
        name = alloc.memorylocations[0].name
        if alloc.kind == "ExternalInput":
            if name != partition_name:
                in_names.append(name)
        elif alloc.kind == "ExternalOutput":
            out_names.append(name)
            out_avals.append(jax.core.ShapedArray(
                tuple(alloc.tensor_shape), mybir.dt.np(alloc.dtype)))
    # The kernel writes every element of "out", so no pre-zeroed donated
    # output buffers are needed — PJRT-allocated (uninit) results are fine.
    all_names = tuple(in_names + ([partition_name] if partition_name else []))

    devices = jax.devices()[:NCORES]
    mesh = Mesh(np.asarray(devices), ("core",))
    repl_sh = NamedSharding(mesh, P())

    def _body(*args):
        operands = list(args)
        if partition_name is not None:
            operands.append(bass2jax.partition_id_tensor())
        return tuple(bass2jax._bass_exec_p.bind(
            *operands,
            out_avals=tuple(out_avals),
            in_names=all_names,
            out_names=tuple(out_names),
            lowering_input_output_aliases=(),
            sim_require_finite=True,
            sim_require_nnan=True,
            nc=nc,
        ))

    in_specs = tuple(P("core") if nm in _DATA_NAMES else P() for nm in in_names)
    out_specs = (P("core"),) * len(out_names)
    fn = jax.jit(
        shard_map(_body, mesh=mesh, in_specs=in_specs, out_specs=out_specs,
                  check_rep=False),
        keep_unused=True)

    runner = dict(fn=fn, devices=devices, in_names=in_names,
                  out_names=out_names, repl_sh=repl_sh,
                  core_sh=NamedSharding(mesh, P("core")), jax=jax, wcache={})
    _RUNNER_CACHE[key] = runner
    return runner


def _hash_arr(arr):
    import hashlib
    a = arr if arr.flags["C_CONTIGUOUS"] else np.ascontiguousarray(arr)
    return hashlib.blake2b(a.data, digest_size=16).digest()


def _kernel_sharded(r, inputs):
    jax, wcache = r["jax"], r["wcache"]
    from concurrent.futures import ThreadPoolExecutor

    pool = r.setdefault("pool", ThreadPoolExecutor(4))

    # convert to fp16 and eagerly start each upload as soon as its
    # conversion finishes — largest tensor first so streaming begins asap
    def conv_put(nm):
        a = np.ascontiguousarray(np.asarray(inputs[nm], dtype=IO_NP))
        return jax.device_put(a, r["core_sh"])
    order = ("point_node", "point_edge", "distribution_edge", "distribution_node")
    futs = {nm: pool.submit(conv_put, nm) for nm in order}

    args = []
    for name in r["in_names"]:
        if name in _DATA_NAMES:
            args.append(futs[name].result())
        else:
            arr = np.ascontiguousarray(np.asarray(inputs[name], dtype=np.float32))
            h = _hash_arr(arr)
            cached = wcache.get(name)
            if cached is None or cached[0] != h:
                cached = (h, jax.device_put(arr, r["repl_sh"]))
                wcache[name] = cached
            args.append(cached[1])
    outs = r["fn"](*args)

    # queue the output downloads before doing any host compute
    shards = None
    try:
        shards = sorted(outs[0].addressable_shards,
                        key=lambda s: s.index[0].start or 0)
        assert len(shards) == NCORES
        for s in shards:
            s.data.copy_to_host_async()
    except Exception:
        shards = None

    # g0 node_l2 = -|vi-vj|^2 of the RAW point_node input — computed on the
    # host (f64 gram to dodge cancellation) while the device round trip and
    # result download are in flight, so it ships zero bytes.
    v = np.asarray(inputs["point_node"], np.float64)
    gram = np.matmul(v, v.transpose(0, 2, 1))
    n2 = np.einsum("bnd,bnd->bn", v, v)
    g0l2 = (2.0 * gram - n2[:, :, None]) - n2[:, None, :]
    idx = np.arange(N)
    g0l2[:, idx, idx] = 0.0

    res = np.empty((G, 3, B, N, N), np.float32)
    res[0, 1] = g0l2
    # device channels: 0=g0 pe, 1=g0 de, 2=g1 pe, 3=g1 l2, 4=g1 de
    chmap = ((0, 0), (0, 2), (1, 0), (1, 1), (1, 2))
    if shards is not None:
        # cast/store each shard as it lands instead of after all arrive
        for c, s in enumerate(shards):
            a = np.asarray(s.data)
            sl = slice(c * BC, (c + 1) * BC)
            for ch, (gg, cc) in enumerate(chmap):
                res[gg, cc, sl] = a[ch]
    else:
        o = np.asarray(outs[0]).reshape(NCORES, 5, BC, N, N)
        for c in range(NCORES):
            sl = slice(c * BC, (c + 1) * BC)
            for ch, (gg, cc) in enumerate(chmap):
                res[gg, cc, sl] = o[c, ch]
    return res


def kernel(**inputs):
    return _kernel_sharded(_get_runner(), inputs)



# revision 35
# speedup vs baseline: 1.0419x; 1.0236x over previous
"""DPGN (gnn_message_passing) fused Trainium2 kernel.

Sharding: pure data parallel over meta-batch B=256 -> 8 cores x 32 samples.
Per core, samples run in 8 blocks of 4. The whole 2-generation DPGN step is
fused on-chip; only inputs/outputs touch HBM.

Layouts (per block of 4 samples b=0..3):
  vT         [128, 120]  point features: channel on partition, (b,i) on free
  d2         [128, 3600] pairwise sq-dists: (b,i,j) on free
  edge tiles [128, 240]  rows 32b+i (32-aligned), free (kk,j)
  dist feats [128, *]    row-group packed: rows 32b+c (c<25)

Runtime: the axon PJRT tunnel has a ~68ms fixed RTT and ~100MB/s streams,
which dominates wall time (HW exec is <8ms).  Hence:
  - the jitted shard_map executable is built ONCE and cached (the stock
    run_bass_kernel_spmd retraces+recompiles per call, ~500ms overhead);
  - replicated weights are content-hashed and live on-device across calls;
  - data inputs and outputs cross the wire as fp16 (tolerance gate 2e-2,
    fp16 wire costs ~1e-3), converted on-chip / on-host;
  - no donated zero output buffers (every output element is written);
  - g0 node_l2 (= -|vi-vj|^2 of the raw point_node input) is recomputed
    on the host in f64 while the device round trip is in flight, so the
    device ships 5 output channels instead of 6.
"""
import sys
sys.path.insert(0, "/opt/trn_rl_repo")
from contextlib import ExitStack

import numpy as np
import concourse.bass as bass
import concourse.bacc as bacc
import concourse.tile as tile
from concourse import mybir
from concourse.bass_utils import run_bass_kernel_spmd
from concourse.masks import make_identity

F32 = mybir.dt.float32
AF = mybir.ActivationFunctionType
OP = mybir.AluOpType
AX = mybir.AxisListType

G, B, N, S, D = 2, 256, 30, 25, 128
NCORES = 8
BC = B // NCORES          # 32 samples per core
NBLK = BC // 4            # 8 blocks of 4 samples
EW = NBLK * N             # 240
NEG = 0.01
BN_SCALE = float(1.0 / np.sqrt(1.0 + 1e-5))
EPS_L1 = 1e-12

# matmul operand mode: "f32" (exact, 4 cyc/row) | "f32r" (reduced-precision mul, 1 cyc/row)
MM_MODE = "f32"
# leaky-relu implementation: "act" (1 ScalarE op; not in CoreSim) | "dve" (Identity + DVE max)
LRELU_ON = "act"
# debug: comma set of enabled parts: "setup,p1,p2,p3,p4,p5" (default all)
import os as _os
PHASES = set((_os.environ.get("KPHASES") or "setup,p1,p2,p3,p4,p5").split(","))
KGENS = int(_os.environ.get("KGENS") or G)
KREPEAT = int(_os.environ.get("KREPEAT") or 1)

_NC_CACHE = {}


BF16 = mybir.dt.bfloat16
F32R = mybir.dt.float32r
F16 = mybir.dt.float16
# wire dtype for the 4 data inputs and the output (tunnel-bandwidth bound;
# tolerance gate is 2e-2, fp16 wire adds ~1e-3)
IO_DT = F16 if (_os.environ.get("KIODT") or "f16") == "f16" else F32
IO_NP = np.float16 if IO_DT == F16 else np.float32


def _dt_point():   # d2, h1, w1T, w2T (base-0 matmuls only)
    if MM_MODE == "hybrid":
        return F32R
    if MM_MODE == "bf16":
        return BF16
    return F32


def _dt_flex():    # dist chain (col/row-tiled matmuls)
    if MM_MODE in ("hybrid", "bf16"):
        return BF16
    return F32


def _dt_s():       # h2 / w3T (s-path: accuracy-sensitive)
    return BF16 if MM_MODE == "bf16" else F32


def _mm(ap):
    return ap


def A(t, ap, off=0):
    return bass.AP(tensor=t.tensor, offset=t.offset + off, ap=ap)


def build_nc():
    nc = bacc.Bacc("TRN2", target_bir_lowering=False, debug=False)
    MDP = _dt_point()
    MDF = _dt_flex()
    MDS = _dt_s()

    pn_d = nc.dram_tensor("point_node", [BC, N, D], IO_DT, kind="ExternalInput")
    pe_d = nc.dram_tensor("point_edge", [BC, N, N], IO_DT, kind="ExternalInput")
    dn_d = nc.dram_tensor("distribution_node", [BC, N, S], IO_DT, kind="ExternalInput")
    de_d = nc.dram_tensor("distribution_edge", [BC, N, N], IO_DT, kind="ExternalInput")
    wd = {}
    for name, shape in [
        ("ps_w1", [G, 2 * D, D]), ("ps_g1", [G, 2 * D]), ("ps_b1", [G, 2 * D]),
        ("ps_w2", [G, D, 2 * D]), ("ps_g2", [G, D]), ("ps_b2", [G, D]),
        ("ps_w3", [G, 1, D]), ("ps_b3", [G, 1]),
        ("p2d_w", [G, S, 2 * S]), ("p2d_b", [G, S]),
        ("ds_w1", [G, 2 * S, S]), ("ds_g1", [G, 2 * S]), ("ds_b1", [G, 2 * S]),
        ("ds_w2", [G, S, 2 * S]), ("ds_g2", [G, S]), ("ds_b2", [G, S]),
        ("ds_w3", [G, 1, S]), ("ds_b3", [G, 1]),
        ("dp_w1", [G, 2 * D, 2 * D]), ("dp_g1", [G, 2 * D]), ("dp_b1", [G, 2 * D]),
        ("dp_w2", [G, D, 2 * D]), ("dp_g2", [G, D]), ("dp_b2", [G, D]),
    ]:
        wd[name] = nc.dram_tensor(name, shape, F32, kind="ExternalInput")
    # output channels: 0=g0 point_edge, 1=g0 dist_edge, 2=g1 point_edge,
    # 3=g1 node_l2, 4=g1 dist_edge.  g0 node_l2 is recomputed on the host
    # from the raw point_node input (cheaper than shipping it).
    out_d = nc.dram_tensor("out", [5, BC, N, N], IO_DT, kind="ExternalOutput")
    OCH, OB = BC * N * N, N * N

    with tile.TileContext(nc) as tc, ExitStack() as ctx:
        cp = ctx.enter_context(tc.tile_pool(name="cpool", bufs=1))
        vp = ctx.enter_context(tc.tile_pool(name="vpool", bufs=1))
        wp = ctx.enter_context(tc.tile_pool(name="wpool", bufs=2))
        ep = ctx.enter_context(tc.tile_pool(name="epool", bufs=2))
        PB = ctx.enter_context(tc.tile_pool(name="PB", bufs=2, space="PSUM"))
        PM = ctx.enter_context(tc.tile_pool(name="PM", bufs=3, space="PSUM"))

        # ================= constants =================
        ident = cp.tile([128, 128], F32, tag="ident")
        make_identity(nc, ident[:])
        off_m = cp.tile([120, N], F32, tag="off_m")           # 1 - eye (30-stride)
        eyeeps = cp.tile([120, N], F32, tag="eyeeps")         # eye + 1e-6
        nc.gpsimd.memset(off_m[:], 1.0)
        nc.gpsimd.memset(eyeeps[:], 1e-6)
        for t, fill in ((off_m, 0.0), (eyeeps, 1.0 + 1e-6)):
            nc.gpsimd.affine_select(
                out=t[0:N, :], in_=t[0:N, :],
                compare_op=OP.not_equal, fill=fill, base=0,
                pattern=[[-1, N]], channel_multiplier=1)
            for b in range(1, 4):
                nc.sync.dma_start(out=t[30 * b:30 * b + N, :], in_=t[0:N, :])
        Eb = cp.tile([S, 4, 128], F32, tag="Eb")              # 1 at (c, 32b+c)
        nc.gpsimd.memset(Eb[:], 0.0)
        for b in range(4):
            nc.gpsimd.affine_select(
                out=Eb[:, b, :], in_=Eb[:, b, :], compare_op=OP.not_equal,
                fill=1.0, base=32 * b, pattern=[[-1, 128]], channel_multiplier=1)
        E2 = cp.tile([2 * S, 2, 128], F32, tag="E2")          # 1 at (c, 64q+c)
        nc.gpsimd.memset(E2[:], 0.0)
        for q in range(2):
            nc.gpsimd.affine_select(
                out=E2[:, q, :], in_=E2[:, q, :], compare_op=OP.not_equal,
                fill=1.0, base=64 * q, pattern=[[-1, 128]], channel_multiplier=1)
        onesT = cp.tile([128, 32], F32, tag="onesT")
        ones_f = cp.tile([128, 32], F32, tag="ones_f")
        nc.vector.memset(ones_f[:], 0.0)
        nc.vector.memset(ones_f[:, 0:1], 1.0)
        nc.vector.tensor_copy(onesT[:], ones_f[:])


        def act_lrelu(out_ap, in_ap, scale, bias):
            if LRELU_ON == "act":
                # Prelu == leaky relu; lives in the sigmoid table set (Lrelu does not,
                # and mixing Lrelu+Sigmoid table loads crashes the ACT engine)
                nc.scalar.activation(out=out_ap, in_=in_ap, func=AF.Prelu,
                                     alpha=NEG, scale=scale, bias=bias)
            elif LRELU_ON == "actsim":
                # timing-equivalent stand-in for CoreSim (values wrong: no lrelu)
                nc.scalar.activation(out=out_ap, in_=in_ap, func=AF.Identity,
                                     scale=scale, bias=bias)
            else:
                nc.scalar.activation(out=out_ap, in_=in_ap, func=AF.Identity,
                                     scale=scale, bias=bias)
                nc.vector.scalar_tensor_tensor(out=out_ap, in0=out_ap, scalar=NEG,
                                               in1=out_ap, op0=OP.mult, op1=OP.max)

        def load_col(name, g, n, tag, blocks=1, scale=None):
            t = cp.tile([128, blocks], F32, tag=tag)
            if blocks > 1:
                src = bass.AP(tensor=wd[name], offset=g * n * blocks,
                              ap=[[1, n], [n, blocks]])
                dst = A(t, [[blocks, n], [1, blocks]])
            else:
                src = bass.AP(tensor=wd[name], offset=g * n, ap=[[1, n]])
                dst = A(t, [[1, n], [1, 1]])
            nc.sync.dma_start(out=dst, in_=src)
            if scale is not None:
                nc.vector.tensor_scalar(out=t[:n, :], in0=t[:n, :], scalar1=scale,
                                        scalar2=None, op0=OP.mult)
            return t

        def load_col_rep(name, g, n, tag, bases, scale=None):
            t = cp.tile([128, 1], F32, tag=tag)
            nc.vector.memset(t[:], 0.0)
            src = bass.AP(tensor=wd[name], offset=g * n, ap=[[1, n], [1, 1]])
            for bb in bases:
                nc.sync.dma_start(out=t[bb:bb + n, :], in_=src)
            if scale is not None:
                for bb in bases:
                    nc.vector.tensor_scalar(out=t[bb:bb + n, :], in0=t[bb:bb + n, :],
                                            scalar1=scale, scalar2=None, op0=OP.mult)
            return t

        def transpose_to(dst_ap, src_ap, idn):
            p = src_ap.partition_size()
            f = src_ap.free_size()
            pt = PM.tile([128, 512], F32, tag="med")
            nc.tensor.transpose(pt[:f, :p], src_ap, idn)
            nc.vector.tensor_copy(dst_ap, pt[:f, :p])

        # ================= weights =================
        W = {g: {} for g in range(G)}
        for g in range(G):
            w = W[g]
            w1T = cp.tile([128, 2 * D], MDP, tag=f"w1T{g}")
            for h in range(2):
                tmp = wp.tile([128, D], F32, tag="wload")
                nc.sync.dma_start(out=tmp[:], in_=wd["ps_w1"][g, 128 * h:128 * (h + 1), :])
                transpose_to(w1T[:, 128 * h:128 * (h + 1)], tmp[:], ident[:])
            w["w1T"] = w1T
            w2T = cp.tile([128, 2, D], MDP, tag=f"w2T{g}")
            tmp = wp.tile([128, 2 * D], F32, tag="wload2")
            nc.sync.dma_start(out=tmp[:], in_=wd["ps_w2"][g])
            for k in range(2):
                transpose_to(w2T[:, k, :], tmp[:, 128 * k:128 * (k + 1)], ident[:])
            w["w2T"] = w2T
            w3T = cp.tile([128, 32], MDS, tag=f"w3T{g}")
            w3f = wp.tile([128, 32], F32, tag="wst")
            nc.vector.memset(w3f[:], 0.0)
            nc.sync.dma_start(out=A(w3f, [[32, 128], [1, 1]]),
                              in_=bass.AP(tensor=wd["ps_w3"], offset=g * D, ap=[[1, D]]))
            nc.vector.tensor_copy(w3T[:], w3f[:])
            w["w3T"] = w3T
            w["gs1"] = load_col("ps_g1", g, 128, f"gs1{g}", 2, scale=BN_SCALE)
            w["bs1"] = load_col("ps_b1", g, 128, f"bs1{g}", 2)
            w["gs2"] = load_col("ps_g2", g, 128, f"gs2{g}", scale=BN_SCALE)
            w["bs2"] = load_col("ps_b2", g, 128, f"bs2{g}")
            b3bc = cp.tile([128, 1], F32, tag=f"b3bc{g}")
            nc.sync.dma_start(out=b3bc[:],
                              in_=bass.AP(tensor=wd["ps_b3"], offset=g, ap=[[0, 128], [1, 1]]))
            w["b3bc"] = b3bc

            tmp = wp.tile([S, 2 * S], F32, tag="wload3")
            nc.sync.dma_start(out=tmp[:], in_=wd["p2d_w"][g])
            p2dA = cp.tile([S, 32], F32, tag=f"p2dA{g}")
            nc.vector.memset(p2dA[:], 0.0)
            transpose_to(p2dA[:, 0:S], tmp[:, 0:S], ident[:S, :S])
            p2dAr = cp.tile([128, 32], F32, tag=f"p2dAr{g}")
            nc.vector.memset(p2dAr[:], 0.0)
            ptA = PM.tile([128, 512], F32, tag="med")
            for b in range(4):
                nc.tensor.matmul(ptA[:, :32], Eb[:, b, :], p2dA[:],
                                 start=(b == 0), stop=(b == 3))
            nc.vector.tensor_copy(p2dAr[:, :], ptA[:, :32])
            w["p2dAr"] = p2dAr
            p2dBf = wp.tile([S, S], F32, tag="p2dBf")
            transpose_to(p2dBf[:], tmp[:, S:2 * S], ident[:S, :S])
            p2dB = cp.tile([128, 32], F32, tag=f"p2dB{g}")
            nc.vector.memset(p2dB[:], 0.0)
            pt = PM.tile([128, 512], F32, tag="med")
            for b in range(4):
                nc.tensor.matmul(pt[:, :S], Eb[:, b, :], p2dBf[:],
                                 start=(b == 0), stop=(b == 3))
            nc.vector.tensor_copy(p2dB[:, 0:S], pt[:, :S])
            w["p2dA"], w["p2dB"] = p2dA, p2dB
            w["p2db"] = load_col_rep("p2d_b", g, S, f"p2db{g}", [0, 32, 64, 96])

            tmp = wp.tile([2 * S, S], F32, tag="wload4")
            nc.sync.dma_start(out=tmp[:], in_=wd["ds_w1"][g])
            dsw1f = wp.tile([S, 2 * S], F32, tag="dsw1f")
            transpose_to(dsw1f[:], tmp[:], ident[:2 * S, :2 * S])
            dsw1 = cp.tile([128, 64], MDF, tag=f"dsw1{g}")
            d1f = wp.tile([128, 64], F32, tag="wst2")
            nc.vector.memset(d1f[:], 0.0)
            pt = PM.tile([128, 512], F32, tag="med")
            for b in range(4):
                nc.tensor.matmul(pt[:, :2 * S], Eb[:, b, :], dsw1f[:],
                                 start=(b == 0), stop=(b == 3))
            nc.vector.tensor_copy(d1f[:, 0:2 * S], pt[:, :2 * S])
            nc.vector.tensor_copy(dsw1[:], d1f[:])
            w["dsw1"] = dsw1
            tmp = wp.tile([S, 2 * S], F32, tag="wload5")
            nc.sync.dma_start(out=tmp[:], in_=wd["ds_w2"][g])
            dsw2f = wp.tile([2 * S, S], F32, tag="dsw2f")
            transpose_to(dsw2f[:], tmp[:], ident[:S, :S])
            dsw2 = cp.tile([128, 32], MDF, tag=f"dsw2{g}")
            d2f = wp.tile([128, 32], F32, tag="wst3")
            nc.vector.memset(d2f[:], 0.0)
            pt = PM.tile([128, 512], F32, tag="med")
            for q in range(2):
                nc.tensor.matmul(pt[:, :S], E2[:, q, :], dsw2f[:],
                                 start=(q == 0), stop=(q == 1))
            nc.vector.tensor_copy(d2f[:, 0:S], pt[:, :S])
            nc.vector.tensor_copy(dsw2[:], d2f[:])
            w["dsw2"] = dsw2
            dsw3 = cp.tile([128, 32], MDF, tag=f"dsw3{g}")
            d3f = wp.tile([128, 32], F32, tag="wst4")
            nc.vector.memset(d3f[:], 0.0)
            for b in range(4):
                nc.sync.dma_start(out=d3f[32 * b:32 * b + S, 0:1],
                                  in_=bass.AP(tensor=wd["ds_w3"], offset=g * S, ap=[[1, S], [1, 1]]))
            nc.vector.tensor_copy(dsw3[:], d3f[:])
            w["dsw3"] = dsw3
            w["dsg1"] = load_col_rep("ds_g1", g, 2 * S, f"dsg1{g}", [0, 64], scale=BN_SCALE)
            w["dsb1"] = load_col_rep("ds_b1", g, 2 * S, f"dsb1{g}", [0, 64])
            w["dsg2"] = load_col_rep("ds_g2", g, S, f"dsg2{g}", [0, 32, 64, 96], scale=BN_SCALE)
            w["dsb2"] = load_col_rep("ds_b2", g, S, f"dsb2{g}", [0, 32, 64, 96])
            dsb3bc = cp.tile([128, 1], F32, tag=f"dsb3bc{g}")
            nc.sync.dma_start(out=dsb3bc[:],
                              in_=bass.AP(tensor=wd["ds_b3"], offset=g, ap=[[0, 128], [1, 1]]))
            w["dsb3bc"] = dsb3bc

            if g < G - 1:
                dpw1T = [cp.tile([128, 2 * D], F32, tag=f"dpw1T{g}_{k}", name=f"dpw1T{g}_{k}") for k in range(2)]
                for r in range(2):
                    tmp = wp.tile([128, 2 * D], F32, tag="wload6")
                    nc.sync.dma_start(out=tmp[:], in_=wd["dp_w1"][g, 128 * r:128 * (r + 1), :])
                    for k in range(2):
                        transpose_to(dpw1T[k][:, 128 * r:128 * (r + 1)],
                                     tmp[:, 128 * k:128 * (k + 1)], ident[:])
                w["dpw1T"] = dpw1T
                tmp = wp.tile([128, 2 * D], F32, tag="wload7")
                nc.sync.dma_start(out=tmp[:], in_=wd["dp_w2"][g])
                dpw2T = [cp.tile([128, D], F32, tag=f"dpw2T{g}_{k}", name=f"dpw2T{g}_{k}") for k in range(2)]
                for k in range(2):
                    transpose_to(dpw2T[k][:], tmp[:, 128 * k:128 * (k + 1)], ident[:])
                w["dpw2T"] = dpw2T
                w["dpg1"] = load_col("dp_g1", g, 128, f"dpg1{g}", 2, scale=BN_SCALE)
                w["dpb1"] = load_col("dp_b1", g, 128, f"dpb1{g}", 2)
                w["dpg2"] = load_col("dp_g2", g, 128, f"dpg2{g}", scale=BN_SCALE)
                w["dpb2"] = load_col("dp_b2", g, 128, f"dpb2{g}")

        # ================= persistent state =================
        vT = [vp.tile([128, BC * N], F32, tag=f"vT{i}", name=f"vT{i}") for i in range(2)]
        dn_rg = vp.tile([128, EW], F32, tag="dn_rg")
        pe_all = vp.tile([120, EW], F32, tag="pe_all")
        de_all = vp.tile([120, EW], F32, tag="de_all")
        s_all = vp.tile([120, EW], F32, tag="s_all")
        sds_all = vp.tile([120, EW], F32, tag="sds_all")
        ef_all = vp.tile([120, EW], F32, tag="ef_all")
        for t in (pe_all, de_all, s_all, sds_all, ef_all, dn_rg, vT[0], vT[1]):
            nc.gpsimd.memset(t[:], 0.0)

        # ---- gen-1 input staging ----
        for kk in range(NBLK):
            pf = wp.tile([120, D], F32, tag="pnflat")
            if IO_DT == F32:
                nc.sync.dma_start(out=pf[:], in_=pn_d[4 * kk:4 * (kk + 1)].rearrange("b n d -> (b n) d"))
            else:
                pf16 = wp.tile([120, D], IO_DT, tag="pnflat16")
                nc.sync.dma_start(out=pf16[:], in_=pn_d[4 * kk:4 * (kk + 1)].rearrange("b n d -> (b n) d"))
                nc.vector.tensor_copy(pf[:], pf16[:])
            pt = PM.tile([128, 512], F32, tag="med")
            nc.tensor.transpose(pt[:, :120], pf[:], ident[:120, :120])
            nc.vector.tensor_copy(vT[0][:, 120 * kk:120 * (kk + 1)], pt[:, :120])

            df = wp.tile([120, S], F32, tag="dnflat")
            if IO_DT == F32:
                nc.sync.dma_start(out=df[:], in_=dn_d[4 * kk:4 * (kk + 1)].rearrange("b n s -> (b n) s"))
            else:
                df16 = wp.tile([120, S], IO_DT, tag="dnflat16")
                nc.sync.dma_start(out=df16[:], in_=dn_d[4 * kk:4 * (kk + 1)].rearrange("b n s -> (b n) s"))
                nc.vector.tensor_copy(df[:], df16[:])
            pt2 = PM.tile([128, 512], F32, tag="med")
            nc.tensor.transpose(pt2[:S, :120], df[:], ident[:120, :120])
            dnf = wp.tile([S, 120], F32, tag="dnf")
            nc.vector.tensor_copy(dnf[:], pt2[:S, :120])
            pt3 = PM.tile([128, 512], F32, tag="med")
            for b in range(4):
                nc.tensor.matmul(pt3[:, :N], Eb[:, b, :], dnf[:, 30 * b:30 * b + N],
                                 start=(b == 0), stop=(b == 3))
            nc.vector.tensor_copy(dn_rg[:, N * kk:N * (kk + 1)], pt3[:, :N])

            for (ed, et) in ((pe_d, pe_all), (de_d, de_all)):
                if IO_DT == F32:
                    nc.sync.dma_start(out=et[:, N * kk:N * (kk + 1)],
                                      in_=ed[4 * kk:4 * (kk + 1)].rearrange("b n m -> (b n) m"))
                else:
                    e16 = wp.tile([120, N], IO_DT, tag="edge16")
                    nc.sync.dma_start(out=e16[:],
                                      in_=ed[4 * kk:4 * (kk + 1)].rearrange("b n m -> (b n) m"))
                    nc.vector.tensor_copy(et[:, N * kk:N * (kk + 1)], e16[:])

        def edge_update(g, w, e_all, sig_src, b3bc, out_ch):
            ssig = ep.tile([120, EW], F32, tag="ssig")
            nc.scalar.activation(out=ssig[:], in_=sig_src[:], func=AF.Sigmoid,
                                 bias=b3bc[:120, :], scale=1.0)
            em = ep.tile([120, EW], F32, tag="em")
            offb = A(off_m, [[N, 120], [0, NBLK], [1, N]])
            emv = A(em, [[EW, 120], [N, NBLK], [1, N]])
            nc.vector.tensor_tensor(out=emv, in0=A(e_all, [[EW, 120], [N, NBLK], [1, N]]),
                                    in1=offb, op=OP.mult)
            esum = ep.tile([120, NBLK], F32, tag="esum")
            nc.vector.tensor_reduce(out=esum[:], in_=emv, axis=AX.X, op=OP.add)
            t = ep.tile([120, EW], F32, tag="t")
            nc.vector.tensor_tensor(out=t[:], in0=ssig[:], in1=em[:], op=OP.mult)
            ts = ep.tile([120, NBLK], F32, tag="ts")
            nc.vector.tensor_reduce(out=ts[:], in_=A(t, [[EW, 120], [N, NBLK], [1, N]]),
                                    axis=AX.X, op=OP.add)
            nc.vector.tensor_scalar(out=ts[:], in0=ts[:], scalar1=EPS_L1,
                                    scalar2=None, op0=OP.max)
            r = ep.tile([120, NBLK], F32, tag="r")
            nc.vector.reciprocal(out=r[:], in_=ts[:])
            nc.vector.tensor_tensor(out=r[:], in0=r[:], in1=esum[:], op=OP.mult)
            e2 = ep.tile([120, EW], F32, tag="e2")
            rb = A(r, [[NBLK, 120], [1, NBLK], [0, N]])
            e2v = A(e2, [[EW, 120], [N, NBLK], [1, N]])
            nc.vector.tensor_tensor(out=e2v, in0=A(t, [[EW, 120], [N, NBLK], [1, N]]),
                                    in1=rb, op=OP.mult)
            eyb = A(eyeeps, [[N, 120], [0, NBLK], [1, N]])
            nc.vector.tensor_tensor(out=e2v, in0=e2v, in1=eyb, op=OP.add)
            rsum = ep.tile([120, NBLK], F32, tag="rsum")
            nc.vector.tensor_reduce(out=rsum[:], in_=e2v, axis=AX.X, op=OP.add)
            rr = ep.tile([120, NBLK], F32, tag="rr")
            nc.vector.reciprocal(out=rr[:], in_=rsum[:])
            rrb = A(rr, [[NBLK, 120], [1, NBLK], [0, N]])
            nc.vector.tensor_tensor(out=A(e_all, [[EW, 120], [N, NBLK], [1, N]]),
                                    in0=e2v, in1=rrb, op=OP.mult)
            if IO_DT == F32:
                esrc = e_all
            else:
                esrc = ep.tile([120, EW], IO_DT, tag="eo16")
                nc.vector.tensor_copy(esrc[:], e_all[:])
            for kk in range(NBLK):
                dst = bass.AP(tensor=out_d,
                              offset=out_ch * OCH + 4 * kk * OB,
                              ap=[[N, 120], [1, N]])
                nc.sync.dma_start(out=dst, in_=esrc[:, N * kk:N * (kk + 1)])

        PSUM_PAT = [[1024, 128], [512, 2], [1, 450]]

        # ================= generations =================
        for _rep in range(KREPEAT):
         for g in range(KGENS):
            w = W[g]
            vc, vn = vT[g % 2], vT[(g + 1) % 2]

            # ---------- phase 1: point sim MLP ----------
            for kk in range(NBLK if "p1" in PHASES else 0):
                base = 120 * kk
                d2 = wp.tile([128, 4 * N * N], MDP, tag="d2")
                vi = A(vc, [[BC * N, 128], [N, 4], [1, N], [0, N]], off=base)
                vj = A(vc, [[BC * N, 128], [N, 4], [0, N], [1, N]], off=base)
                dv = A(d2, [[3600, 128], [900, 4], [N, N], [1, N]])
                nc.vector.tensor_tensor(out=dv, in0=vi, in1=vj, op=OP.subtract)
                nc.vector.tensor_tensor(out=d2[:], in0=d2[:], in1=d2[:], op=OP.mult)
                h2 = wp.tile([128, 4 * N * N], MDS, tag="h2")
                for bb in range(4):   # per sample
                    h1 = [wp.tile([128, N * N], MDP, tag=f"h1_{h}", name=f"h1_{h}") for h in range(2)]
                    for h in range(2):
                        pb = PB.tile([128, 2, 512], F32, tag="big")
                        for p in range(2):
                            nc.tensor.matmul(pb[:, p, 0:450],
                                             _mm(w["w1T"][:, 128 * h:128 * (h + 1)]),
                                             _mm(d2[:, 900 * bb + 450 * p:900 * bb + 450 * (p + 1)]),
                                             start=True, stop=True)
                        act_lrelu(A(h1[h], [[900, 128], [450, 2], [1, 450]]),
                                  A(pb, PSUM_PAT),
                                  w["gs1"][:, h:h + 1], w["bs1"][:, h:h + 1])
                    pb = PB.tile([128, 2, 512], F32, tag="big")
                    for p in range(2):
                        for k in range(2):
                            nc.tensor.matmul(pb[:, p, 0:450],
                                             _mm(w["w2T"][:, k, :]),
                                             _mm(h1[k][:, 450 * p:450 * (p + 1)]),
                                             start=(k == 0), stop=(k == 1))
                    act_lrelu(A(h2, [[3600, 128], [450, 2], [1, 450]], off=900 * bb),
                              A(pb, PSUM_PAT), w["gs2"][:], w["bs2"][:])
                # s_pre and node_l2 via col-tiled M=1 matmuls
                # (g0 node_l2 is host-computed from raw point_node — skip stage 1)
                for stage in range(2 if g == 1 else 1):
                    rhs_t, lhs = (h2, w["w3T"]) if stage == 0 else (d2, onesT)
                    pb = PB.tile([128, 2, 512], F32, tag="big")
                    for p in range(2):
                        for b in range(4):
                            rr = rhs_t[:, 900 * b + 450 * p:900 * b + 450 * (p + 1)]
                            if stage == 1 and rr.dtype == F32R:
                                rr = rr.bitcast(F32)
                            nc.tensor.matmul(
                                pb[32 * b:32 * b + 32, p, 0:450],
                                lhs[:], rr,
                                start=True, stop=True, tile_position=(0, 32 * b))
                    stg = wp.tile([128, 900], F32, tag=f"stg{stage}")
                    if stage == 0:
                        nc.vector.tensor_copy(A(stg, [[900, 128], [450, 2], [1, 450]]),
                                              A(pb, PSUM_PAT))
                        src = A(stg, [[32 * 900, 4], [N, N], [1, N]])
                        nc.sync.dma_start(out=s_all[:, N * kk:N * (kk + 1)], in_=src)
                    else:
                        if IO_DT != F32:
                            stgo = wp.tile([128, 900], IO_DT, tag="stg1o", name="stgo")
                        else:
                            stgo = stg
                        nc.vector.tensor_scalar(
                            out=A(stgo, [[900, 128], [450, 2], [1, 450]]),
                            in0=A(pb, PSUM_PAT),
                            scalar1=-1.0, scalar2=None, op0=OP.mult)
                        for b in range(4):
                            src = A(stgo, [[900, 1], [N, N], [1, N]], off=32 * b * 900)
                            dst = bass.AP(tensor=out_d,
                                          offset=3 * OCH + (4 * kk + b) * OB,
                                          ap=[[N, N], [1, N]])
                            nc.sync.dma_start(out=dst, in_=src)

            # ---------- phase 2: point edge update ----------
            if "p2" in PHASES:
                edge_update(g, w, pe_all, s_all, w["b3bc"], 0 if g == 0 else 2)

            # ---------- phase 3: p2d + dist sim ----------
            for kk in range(NBLK if "p3" in PHASES else 0):
                peT = wp.tile([S, 120], F32, tag="peT")
                pt = PM.tile([128, 512], F32, tag="med")
                nc.tensor.transpose(pt[:S, :120], pe_all[:, N * kk:N * kk + S],
                                    ident[:120, :120])
                nc.vector.tensor_copy(peT[:], pt[:S, :120])
                ptg = PM.tile([128, 512], F32, tag="med")
                for b in range(4):
                    nc.tensor.matmul(ptg[:, :N], Eb[:, b, :],
                                     peT[:, 30 * b:30 * b + N],
                                     start=(b == 0), stop=(b == 3))
                peRG = wp.tile([128, N], F32, tag="peRG")
                nc.vector.tensor_copy(peRG[:], ptg[:, :N])
                pg = PM.tile([128, 512], F32, tag="med")
                for b in range(4):
                    nc.tensor.matmul(pg[32 * b:32 * b + 32, :N],
                                     _mm(w["p2dAr"][32 * b:32 * b + S, :]),
                                     _mm(peRG[32 * b:32 * b + S, :]),
                                     start=True, stop=False, tile_position=(32 * b, 32 * b))
                    nc.tensor.matmul(pg[32 * b:32 * b + 32, :N],
                                     _mm(w["p2dB"][32 * b:32 * b + S, :]),
                                     _mm(dn_rg[32 * b:32 * b + S, N * kk:N * (kk + 1)]),
                                     start=False, stop=True, tile_position=(32 * b, 32 * b))
                act_lrelu(dn_rg[:, N * kk:N * (kk + 1)], pg[:, :N], 1.0, w["p2db"][:])
                dd2 = wp.tile([128, N * N], MDF, tag="dd2")
                vi = A(dn_rg, [[EW, 128], [1, N], [0, N]], off=N * kk)
                vj = A(dn_rg, [[EW, 128], [0, N], [1, N]], off=N * kk)
                nc.vector.tensor_tensor(out=A(dd2, [[900, 128], [N, N], [1, N]]),
                                        in0=vi, in1=vj, op=OP.subtract)
                nc.vector.tensor_tensor(out=dd2[:], in0=dd2[:], in1=dd2[:], op=OP.mult)
                h1d = [wp.tile([128, N * N], MDF, tag=f"h1d{p}", name=f"h1d{p}") for p in range(2)]
                for pair in range(2):
                    pb = PB.tile([128, 2, 512], F32, tag="big")
                    for ck in range(2):
                        for q in range(2):
                            b = 2 * pair + q
                            nc.tensor.matmul(
                                pb[64 * q:64 * q + 64, ck, 0:450],
                                _mm(w["dsw1"][32 * b:32 * b + S, :]),
                                _mm(dd2[32 * b:32 * b + S, 450 * ck:450 * (ck + 1)]),
                                start=True, stop=True, tile_position=(32 * b, 64 * q))
                    act_lrelu(A(h1d[pair], [[900, 128], [450, 2], [1, 450]]),
                              A(pb, PSUM_PAT), w["dsg1"][:], w["dsb1"][:])
                h2d = wp.tile([128, N * N], MDF, tag="h2d")
                pb = PB.tile([128, 2, 512], F32, tag="big")
                for ck in range(2):
                    for pair in range(2):
                        for q in range(2):
                            b = 2 * pair + q
                            nc.tensor.matmul(
                                pb[32 * b:32 * b + 32, ck, 0:450],
                                _mm(w["dsw2"][64 * q:64 * q + 2 * S, :]),
                                _mm(h1d[pair][64 * q:64 * q + 2 * S, 450 * ck:450 * (ck + 1)]),
                                start=True, stop=True, tile_position=(64 * q, 32 * b))
                act_lrelu(A(h2d, [[900, 128], [450, 2], [1, 450]]),
                          A(pb, PSUM_PAT), w["dsg2"][:], w["dsb2"][:])
                pb = PB.tile([128, 2, 512], F32, tag="big")
                for ck in range(2):
                    for b in range(4):
                        nc.tensor.matmul(
                            pb[32 * b:32 * b + 32, ck, 0:450],
                            _mm(w["dsw3"][32 * b:32 * b + S, :]),
                            _mm(h2d[32 * b:32 * b + S, 450 * ck:450 * (ck + 1)]),
                            start=True, stop=True, tile_position=(32 * b, 32 * b))
                stg = wp.tile([128, 900], F32, tag="stgd")
                nc.vector.tensor_copy(A(stg, [[900, 128], [450, 2], [1, 450]]),
                                      A(pb, PSUM_PAT))
                src = A(stg, [[32 * 900, 4], [N, N], [1, N]])
                nc.sync.dma_start(out=sds_all[:, N * kk:N * (kk + 1)], in_=src)

            # ---------- phase 4: dist edge update (+ ef) ----------
            if "p4" in PHASES:
                edge_update(g, w, de_all, sds_all, w["dsb3bc"], 1 if g == 0 else 4)
            if g < G - 1 and "p5" in PHASES:
                em2 = ep.tile([120, EW], F32, tag="em2")
                offb = A(off_m, [[N, 120], [0, NBLK], [1, N]])
                em2v = A(em2, [[EW, 120], [N, NBLK], [1, N]])
                nc.vector.tensor_tensor(out=em2v,
                                        in0=A(de_all, [[EW, 120], [N, NBLK], [1, N]]),
                                        in1=offb, op=OP.mult)
                s2 = ep.tile([120, NBLK], F32, tag="s2")
                nc.vector.tensor_reduce(out=s2[:], in_=em2v, axis=AX.X, op=OP.add)
                nc.vector.tensor_scalar(out=s2[:], in0=s2[:], scalar1=EPS_L1,
                                        scalar2=None, op0=OP.max)
                r2 = ep.tile([120, NBLK], F32, tag="r2")
                nc.vector.reciprocal(out=r2[:], in_=s2[:])
                r2b = A(r2, [[NBLK, 120], [1, NBLK], [0, N]])
                nc.vector.tensor_tensor(out=A(ef_all, [[EW, 120], [N, NBLK], [1, N]]),
                                        in0=em2v, in1=r2b, op=OP.mult)

                # ---------- phase 5: d2p ----------
                for kk in range(NBLK):
                    base = 120 * kk
                    efT = wp.tile([N, 120], F32, tag="efT")
                    pt = PM.tile([128, 512], F32, tag="med")
                    nc.tensor.transpose(pt[:N, :120],
                                        ef_all[:, N * kk:N * (kk + 1)], ident[:120, :120])
                    nc.vector.tensor_copy(efT[:], pt[:N, :120])
                    pnat = wp.tile([N, 4 * D], F32, tag="pnat")
                    pt2 = PM.tile([128, 512], F32, tag="med")
                    for b in range(4):
                        nc.tensor.transpose(pt2[:N, 128 * b:128 * (b + 1)],
                                            vc[:, base + 30 * b:base + 30 * b + N],
                                            ident[:])
                    nc.vector.tensor_copy(pnat[:], pt2[:N, :])
                    pag = PM.tile([128, 512], F32, tag="med")
                    for b in range(4):
                        nc.tensor.matmul(pag[:, 30 * b:30 * b + N],
                                         _mm(pnat[:, 128 * b:128 * (b + 1)]),
                                         _mm(efT[:, 30 * b:30 * b + N]),
                                         start=True, stop=True)
                    aggr = wp.tile([128, 120], F32, tag="aggr")
                    nc.vector.tensor_copy(aggr[:], pag[:, :120])
                    hdp = [wp.tile([128, 120], F32, tag=f"hdp{h}", name=f"hdp{h}") for h in range(2)]
                    for h in range(2):
                        pm_ = PM.tile([128, 512], F32, tag="med")
                        nc.tensor.matmul(pm_[:, :120],
                                         _mm(w["dpw1T"][0][:, 128 * h:128 * (h + 1)]),
                                         _mm(vc[:, base:base + 120]),
                                         start=True, stop=False)
                        nc.tensor.matmul(pm_[:, :120],
                                         _mm(w["dpw1T"][1][:, 128 * h:128 * (h + 1)]),
                                         _mm(aggr[:]), start=False, stop=True)
                        act_lrelu(hdp[h][:], pm_[:, :120],
                                  w["dpg1"][:, h:h + 1], w["dpb1"][:, h:h + 1])
                    pm_ = PM.tile([128, 512], F32, tag="med")
                    for k in range(2):
                        nc.tensor.matmul(pm_[:, :120], _mm(w["dpw2T"][k][:]),
                                         _mm(hdp[k][:]), start=(k == 0), stop=(k == 1))
                    act_lrelu(vn[:, base:base + 120], pm_[:, :120],
                              w["dpg2"][:], w["dpb2"][:])

    nc.compile()
    return nc


def _get_nc():
    key = MM_MODE
    if key not in _NC_CACHE:
        _NC_CACHE[key] = build_nc()
    return _NC_CACHE[key]


# ---------------------------------------------------------------------------
# Cached PJRT runner.  run_bass_kernel_spmd builds a fresh jax.jit closure on
# every call (full retrace + XLA/PJRT compile + 8x replicated weight upload
# per invocation).  Here the jitted shard_map executable is built once and
# the replicated weights live on-device across calls.
# ---------------------------------------------------------------------------
_RUNNER_CACHE = {}
_DATA_NAMES = ("point_node", "point_edge", "distribution_node", "distribution_edge")


def _get_runner():
    key = MM_MODE
    if key in _RUNNER_CACHE:
        return _RUNNER_CACHE[key]
    import jax
    from jax.experimental.shard_map import shard_map
    from jax.sharding import Mesh, NamedSharding, PartitionSpec as P
    from concourse import bass2jax

    nc = _get_nc()
    bass2jax.install_neuronx_cc_hook()
    assert nc.dbg_addr is None, "rebuild with debug=False"
    partition_name = nc.partition_id_tensor.name if nc.partition_id_tensor else None

    in_names, out_names, out_avals = [], [], []
    for alloc in nc.m.functions[0].allocations:
        if not isinstance(alloc, mybir.MemoryLocationSet):
            continue
        name = alloc.memorylocations[0].name
        if alloc.kind == "ExternalInput":
            if name != partition_name:
                in_names.append(name)
        elif alloc.kind == "ExternalOutput":
            out_names.append(name)
            out_avals.append(jax.core.ShapedArray(
                tuple(alloc.tensor_shape), mybir.dt.np(alloc.dtype)))
    # The kernel writes every element of "out", so no pre-zeroed donated
    # output buffers are needed — PJRT-allocated (uninit) results are fine.
    all_names = tuple(in_names + ([partition_name] if partition_name else []))

    devices = jax.devices()[:NCORES]
    mesh = Mesh(np.asarray(devices), ("core",))
    repl_sh = NamedSharding(mesh, P())

    def _body(*args):
        operands = list(args)
        if partition_name is not None:
            operands.append(bass2jax.partition_id_tensor())
        return tuple(bass2jax._bass_exec_p.bind(
            *operands,
            out_avals=tuple(out_avals),
            in_names=all_names,
            out_names=tuple(out_names),
            lowering_input_output_aliases=(),
            sim_require_finite=True,
            sim_require_nnan=True,
            nc=nc,
        ))

    in_specs = tuple(P("core") if nm in _DATA_NAMES else P() for nm in in_names)
    out_specs = (P("core"),) * len(out_names)
    fn = jax.jit(
        shard_map(_body, mesh=mesh, in_specs=in_specs, out_specs=out_specs,
                  check_rep=False),
        keep_unused=True)

    runner = dict(fn=fn, devices=devices, in_names=in_names,
                  out_names=out_names, repl_sh=repl_sh,
                  core_sh=NamedSharding(mesh, P("core")), jax=jax, wcache={})
    _RUNNER_CACHE[key] = runner
    return runner


def _hash_arr(arr):
    import hashlib
    a = arr if arr.flags["C_CONTIGUOUS"] else np.ascontiguousarray(arr)
    return hashlib.blake2b(a.data, digest_size=16).digest()


def _kernel_sharded(r, inputs):
    jax, wcache = r["jax"], r["wcache"]
    from concurrent.futures import ThreadPoolExecutor

    pool = r.setdefault("pool", ThreadPoolExecutor(4))

    # convert to fp16 in worker threads, then eagerly start each upload from
    # the main thread as its conversion finishes — largest tensor first so
    # streaming begins asap (device_put is kept single-threaded on purpose)
    order = ("point_node", "point_edge", "distribution_edge", "distribution_node")
    futs = {nm: pool.submit(
        lambda n=nm: np.ascontiguousarray(np.asarray(inputs[n], dtype=IO_NP)))
        for nm in order}
    data_dev = {nm: jax.device_put(futs[nm].result(), r["core_sh"])
                for nm in order}

    args = []
    for name in r["in_names"]:
        if name in _DATA_NAMES:
            args.append(data_dev[name])
        else:
            arr = np.ascontiguousarray(np.asarray(inputs[name], dtype=np.float32))
            h = _hash_arr(arr)
            cached = wcache.get(name)
            if cached is None or cached[0] != h:
                cached = (h, jax.device_put(arr, r["repl_sh"]))
                wcache[name] = cached
            args.append(cached[1])
    outs = r["fn"](*args)

    # queue the output downloads before doing any host compute
    shards = None
    try:
        shards = sorted(outs[0].addressable_shards,
                        key=lambda s: s.index[0].start or 0)
        assert len(shards) == NCORES
        for s in shards:
            s.data.copy_to_host_async()
    except Exception:
        shards = None

    # g0 node_l2 = -|vi-vj|^2 of the RAW point_node input — computed on the
    # host (f64 gram to dodge cancellation) while the device round trip and
    # result download are in flight, so it ships zero bytes.
    v = np.asarray(inputs["point_node"], np.float64)
    gram = np.matmul(v, v.transpose(0, 2, 1))
    n2 = np.einsum("bnd,bnd->bn", v, v)
    g0l2 = (2.0 * gram - n2[:, :, None]) - n2[:, None, :]
    idx = np.arange(N)
    g0l2[:, idx, idx] = 0.0

    res = np.empty((G, 3, B, N, N), np.float32)
    res[0, 1] = g0l2
    # device channels: 0=g0 pe, 1=g0 de, 2=g1 pe, 3=g1 l2, 4=g1 de
    chmap = ((0, 0), (0, 2), (1, 0), (1, 1), (1, 2))
    if shards is not None:
        # cast/store each shard as it lands instead of after all arrive
        for c, s in enumerate(shards):
            a = np.asarray(s.data)
            sl = slice(c * BC, (c + 1) * BC)
            for ch, (gg, cc) in enumerate(chmap):
                res[gg, cc, sl] = a[ch]
    else:
        o = np.asarray(outs[0]).reshape(NCORES, 5, BC, N, N)
        for c in range(NCORES):
            sl = slice(c * BC, (c + 1) * BC)
            for ch, (gg, cc) in enumerate(chmap):
                res[gg, cc, sl] = o[c, ch]
    return res


def kernel(**inputs):
    return _kernel_sharded(_get_runner(), inputs)

